# revision 1
# baseline (speedup 1.0000x reference)
"""Trainium2 Bass kernel for nn_DiscreteDecisionTransformer.

Decision-transformer forward: embed(a,r,s) -> LN -> +posenc, then 4 blocks of
[causal self-attn, cross-attn, FFN] with post-LN, then action head.

Distribution: data-parallel over batch, 16 batches / 8 cores = 2 per core.
Params replicated; zero collectives. Inside each core everything is
feature-major ([dmodel on partitions, tokens on free dim]) so GEMMs contract
over partitions with no transposes.

Key simplifications baked into the host prep:
 - Cross-attention has a single key/value (one task token), so softmax==1 and
   the whole cross-attn block collapses to a per-(block,batch) bias vector,
   precomputed on host and fused into LN1's beta.
 - Q-side 1/sqrt(dh) folded into Wq/bq.
 - Causal mask is additive (-30000 on the 4 diagonal-band tiles); fully
   masked key tiles are skipped outright.
 - Softmax denominators come free from the PV matmul via a ones-column
   appended to V (97-column heads); no max-subtraction needed (scores are
   O(few) by construction, exp never overflows).
 - LayerNorm stats (sum, sum-of-squares) are cross-partition reductions done
   on the PE with a ones-vector lhsT; per-token scale A=rstd and shift
   B=mu*rstd are broadcast across partitions on GpSimd.

GEMMs run in bf16 with f32 PSUM accumulation (fp32 matmul is 4x slower and
float32r locks up the device); measured end-to-end error vs the f32 reference
is <1e-2 scale-relative.
"""

import sys
from contextlib import ExitStack

sys.path.insert(0, "/opt/trn_rl_repo")

import numpy as np
import ml_dtypes

import concourse.bacc as bacc
import concourse.mybir as mybir
import concourse.tile as tile
from concourse.bass_utils import run_bass_kernel_spmd

bf = ml_dtypes.bfloat16

B, L, D, H, DH, NB, E = 16, 1024, 768, 8, 96, 4, 256
A_DIM, S_DIM = 64, 128
NCORES = 8
CPC = B // NCORES  # batches per core
KT = D // 128      # 6 k-tiles of dmodel
MT = D // 128      # 6 m-tiles of dmodel
CH = 512           # token chunk (matmul N)
NCH = L // CH      # 2 chunks per batch
FFT = 4 * D // 128 # 24 m-tiles of ffn hidden
F32, BF = mybir.dt.float32, mybir.dt.bfloat16
AL = mybir.AluOpType
AF = mybir.ActivationFunctionType

_CACHE = {}


def _rearr_pk(ap, p):
    return ap.rearrange("(k p) -> p k", p=p)


def _build(reps=1):
    """Emit the full per-core program. Returns the finished Bacc object."""
    nc = bacc.Bacc("TRN2", target_bir_lowering=False, debug=False)
    dram = nc.dram_tensor

    ars = dram("ars", [CPC, 193, L], BF, kind="ExternalInput")
    wa = dram("wa", [A_DIM, E], BF, kind="ExternalInput")
    wr = dram("wr", [1, E], BF, kind="ExternalInput")
    ws = dram("ws", [S_DIM, E], BF, kind="ExternalInput")
    bemb = dram("bemb", [D], F32, kind="ExternalInput")
    lnp0 = dram("lnp0", [3, D], F32, kind="ExternalInput")
    pos = dram("pos", [D, L], F32, kind="ExternalInput")
    wq = dram("wq", [NB, D, D], BF, kind="ExternalInput")
    wk = dram("wk", [NB, D, D], BF, kind="ExternalInput")
    wv = dram("wv", [NB, D, D], BF, kind="ExternalInput")
    wo = dram("wo", [NB, D, D], BF, kind="ExternalInput")
    w1 = dram("w1", [NB, D, 4 * D], BF, kind="ExternalInput")
    w2 = dram("w2", [NB, 4 * D, D], BF, kind="ExternalInput")
    bq = dram("bq", [NB, D], F32, kind="ExternalInput")
    bk = dram("bk", [NB, D], F32, kind="ExternalInput")
    bvb = dram("bvb", [NB, 128, 8 * 97], BF, kind="ExternalInput")
    bo = dram("bo", [NB, D], F32, kind="ExternalInput")
    b1 = dram("b1", [NB, 4 * D], F32, kind="ExternalInput")
    b2 = dram("b2", [NB, D], F32, kind="ExternalInput")
    cabb = dram("cabb", [NB, CPC, D], F32, kind="ExternalInput")
    ln1g = dram("ln1g", [NB, 2, D], F32, kind="ExternalInput")
    lnp = dram("lnp", [NB, 2, 3, D], F32, kind="ExternalInput")
    masks = dram("masks", [128, 896], BF, kind="ExternalInput")
    fcw = dram("fcw", [D, A_DIM], BF, kind="ExternalInput")
    fcb = dram("fcb", [A_DIM], F32, kind="ExternalInput")
    y = dram("y", [CPC, A_DIM, L], F32, kind="ExternalOutput")

    with nc.allow_low_precision(reason="bf16 kernel by design"), \
         tile.TileContext(nc) as tc, ExitStack() as ctx:
            ep = ctx.enter_context
            cst = ep(tc.tile_pool(name="cst", bufs=1))
            wblk = ep(tc.tile_pool(name="wblk", bufs=1))
            wstr = ep(tc.tile_pool(name="wstr", bufs=8))
            w2str = ep(tc.tile_pool(name="w2str", bufs=3))
            xp = ep(tc.tile_pool(name="xp", bufs=1))
            qkp = ep(tc.tile_pool(name="qk", bufs=1))
            vap = ep(tc.tile_pool(name="vap", bufs=1))
            ptp = ep(tc.tile_pool(name="ptp", bufs=8))
            otp = ep(tc.tile_pool(name="otp", bufs=1))
            scr = ep(tc.tile_pool(name="scr", bufs=3))
            hp = ep(tc.tile_pool(name="hp", bufs=1))
            smv = ep(tc.tile_pool(name="smv", bufs=3))
            abp = ep(tc.tile_pool(name="abp", bufs=1))
            bias = ep(tc.tile_pool(name="bias", bufs=1))
            pmm = ep(tc.tile_pool(name="pmm", bufs=5, space="PSUM"))
            ppv = ep(tc.tile_pool(name="ppv", bufs=2, space="PSUM"))
            pst = ep(tc.tile_pool(name="pst", bufs=1, space="PSUM"))
            # ---------- global constants ----------
            ones = cst.tile([128, 1], BF)
            nc.any.memset(ones[:], 1.0)
            epst = cst.tile([1, 1], F32)
            nc.any.memset(epst[:], 1e-5)
            bigm = cst.tile([128, 896], BF, tag="bigm")
            nc.sync.dma_start(bigm[:], masks[:])
            maskt = [bigm[:, 384 - rt * 128:896 - rt * 128] for rt in range(4)]
            fct = []
            for k in range(KT):
                t = cst.tile([128, A_DIM], BF, tag=f"fcw{k}")
                nc.sync.dma_start(t[:], fcw[k * 128:(k + 1) * 128, :])
                fct.append(t)
            fcbt = cst.tile([A_DIM, 1], F32, tag="fcb")
            nc.sync.dma_start(fcbt[:], fcb[:].rearrange("(m o) -> m o", o=1))

            # residual-stream tiles, two roles that alternate per LN
            xt = [[[xp.tile([128, L], BF, tag=f"x{b}_{j}_{k}", name=f"x{b}_{j}_{k}") for k in range(KT)]
                   for j in range(2)] for b in range(CPC)]

            def ln_chunk(b, c, IN, OUT, gt, gnt, bt_, post_pos=False):
                """LayerNorm over features for one 512-token chunk.

                IN/OUT: lists of 6 [128, L] bf16 tiles (feature-major).
                gt/gnt/bt_: [128, 6] param tiles (gamma, -gamma, beta).
                post_pos: add positional-encoding chunk after the affine step.
                """
                cs = slice(c * CH, (c + 1) * CH)
                st = pst.tile([33, CH], F32, tag="st")
                for k in range(KT):
                    nc.tensor.matmul(st[0:1, :], ones[:], IN[k][:, cs],
                                     start=(k == 0), stop=(k == KT - 1))
                for k in range(KT):
                    xsq = scr.tile([128, CH], BF, tag="xsq", bufs=1)
                    nc.scalar.activation(xsq[:], IN[k][:, cs], AF.Square)
                    nc.tensor.matmul(st[32:33, :], ones[:], xsq[:],
                                     start=(k == 0), stop=(k == KT - 1))
                mu = smv.tile([1, CH], F32, tag="mu", bufs=1)
                nc.vector.tensor_scalar_mul(mu[:], st[0:1, :], 1.0 / D)
                m2 = smv.tile([1, CH], F32, tag="sm")
                nc.vector.tensor_scalar_mul(m2[:], st[32:33, :], 1.0 / D)
                mu2 = smv.tile([1, CH], F32, tag="sm")
                nc.vector.tensor_mul(mu2[:], mu[:], mu[:])
                var = smv.tile([1, CH], F32, tag="sm")
                nc.vector.tensor_sub(var[:], m2[:], mu2[:])
                sd = smv.tile([1, CH], F32, tag="sm")
                nc.scalar.activation(sd[:], var[:], AF.Sqrt, bias=epst[:])
                ab = abp.tile([1, 2 * CH], BF, tag="ab")
                nc.vector.reciprocal(ab[:, 0:CH], sd[:])
                nc.vector.tensor_mul(ab[:, CH:2 * CH], mu[:], ab[:, 0:CH])
                abb = abp.tile([128, 2 * CH], BF, tag="abb")
                nc.gpsimd.partition_broadcast(abb[:], ab[:])
                for k in range(KT):
                    u = scr.tile([128, CH], F32, tag="scr")
                    nc.vector.scalar_tensor_tensor(
                        u[:], IN[k][:, cs], gt[:, k:k + 1], abb[:, 0:CH],
                        op0=AL.mult, op1=AL.mult)
                    w_ = scr.tile([128, CH], F32, tag="scr")
                    nc.vector.scalar_tensor_tensor(
                        w_[:], abb[:, CH:2 * CH], gnt[:, k:k + 1], u[:],
                        op0=AL.mult, op1=AL.add)
                    if post_pos:
                        t2 = scr.tile([128, CH], F32, tag="scr")
                        nc.scalar.activation(t2[:], w_[:], AF.Identity,
                                             bias=bt_[:, k:k + 1])
                        pe = scr.tile([128, CH], F32, tag="scr")
                        nc.sync.dma_start(pe[:], pos[k * 128:(k + 1) * 128, cs])
                        nc.vector.tensor_add(OUT[k][:, cs], t2[:], pe[:])
                    else:
                        nc.scalar.activation(OUT[k][:, cs], w_[:], AF.Identity,
                                             bias=bt_[:, k:k + 1])

            def emit_forward():
                # ---------- embed + LN + posenc ----------
                wat = cst.tile([A_DIM, E], BF, tag="wa")
                nc.sync.dma_start(wat[:], wa[:])
                wrt = cst.tile([1, E], BF, tag="wr")
                nc.sync.dma_start(wrt[:], wr[:])
                wst = cst.tile([S_DIM, E], BF, tag="ws")
                nc.sync.dma_start(wst[:], ws[:])
                bembt = cst.tile([128, KT], F32, tag="bemb")
                nc.sync.dma_start(bembt[:], _rearr_pk(bemb[:], 128))
                p0 = []
                for j in range(3):
                    t = cst.tile([128, KT], F32, tag=f"lnp0{j}")
                    nc.sync.dma_start(t[:], _rearr_pk(lnp0[j], 128))
                    p0.append(t)

                for b in range(CPC):
                    for c in range(NCH):
                        cs = slice(c * CH, (c + 1) * CH)
                        ta = scr.tile([A_DIM, CH], BF, tag="scr")
                        nc.sync.dma_start(ta[:], ars[b, 0:A_DIM, cs])
                        tr = scr.tile([1, CH], BF, tag="scr")
                        nc.sync.dma_start(tr[:], ars[b, A_DIM:A_DIM + 1, cs])
                        ts = scr.tile([S_DIM, CH], BF, tag="scr")
                        nc.sync.dma_start(ts[:], ars[b, A_DIM + 1:193, cs])
                        for m in range(MT):
                            p = pmm.tile([128, CH], F32, tag="mm")
                            ms = slice((m % 2) * 128, (m % 2) * 128 + 128)
                            if m < 2:
                                nc.tensor.matmul(p[:], wat[:, ms], ta[:],
                                                 start=True, stop=True)
                            elif m < 4:
                                nc.tensor.matmul(p[:], wrt[:, ms], tr[:],
                                                 start=True, stop=True)
                            else:
                                nc.tensor.matmul(p[:], wst[:, ms], ts[:],
                                                 start=True, stop=True)
                            nc.vector.tensor_scalar_add(xt[b][0][m][:, cs], p[:],
                                                        bembt[:, m:m + 1])
                        ln_chunk(b, c, xt[b][0], xt[b][1], p0[0], p0[1], p0[2],
                                 post_pos=True)

                # roles: after embed, x lives in role 1
                cur = [1, 1]

                # ---------- transformer blocks ----------
                for i in range(NB):
                    wqt, wkt, wvt = [], [], []
                    for k in range(KT):
                        ks = slice(k * 128, (k + 1) * 128)
                        for lst, src, tag in ((wqt, wq, "wq"), (wkt, wk, "wk"),
                                              (wvt, wv, "wv")):
                            t = wblk.tile([128, D], BF, tag=f"{tag}{k}")
                            nc.sync.dma_start(t[:], src[i, ks, :])
                            lst.append(t)
                    bqt = bias.tile([DH, H], F32, tag="bq")
                    nc.sync.dma_start(bqt[:], _rearr_pk(bq[i], DH))
                    bkt = bias.tile([DH, H], F32, tag="bk")
                    nc.sync.dma_start(bkt[:], _rearr_pk(bk[i], DH))
                    bvbt = bias.tile([128, 8 * 97], BF, tag="bvb")
                    nc.sync.dma_start(bvbt[:], bvb[i])
                    bot = bias.tile([128, MT], F32, tag="bo")
                    nc.sync.dma_start(bot[:], _rearr_pk(bo[i], 128))
                    b1t = bias.tile([128, FFT], F32, tag="b1")
                    nc.sync.dma_start(b1t[:], _rearr_pk(b1[i], 128))
                    b2t = bias.tile([128, MT], F32, tag="b2")
                    nc.sync.dma_start(b2t[:], _rearr_pk(b2[i], 128))
                    cabt = []
                    for b in range(CPC):
                        t = bias.tile([128, KT], F32, tag=f"cab{b}")
                        nc.sync.dma_start(t[:], _rearr_pk(cabb[i, b], 128))
                        cabt.append(t)
                    l1g = bias.tile([128, KT], F32, tag="l1g")
                    nc.sync.dma_start(l1g[:], _rearr_pk(ln1g[i, 0], 128))
                    l1n = bias.tile([128, KT], F32, tag="l1n")
                    nc.sync.dma_start(l1n[:], _rearr_pk(ln1g[i, 1], 128))
                    lp = {}
                    for li, lname in ((0, "l2"), (1, "l3")):
                        for j, jn in ((0, "g"), (1, "n"), (2, "b")):
                            t = bias.tile([128, KT], F32, tag=f"{lname}{jn}")
                            nc.sync.dma_start(t[:], _rearr_pk(lnp[i, li, j], 128))
                            lp[f"{lname}{jn}"] = t

                    for b in range(CPC):
                        X = xt[b][cur[b]]          # block input (role j)
                        R = xt[b][1 - cur[b]]      # scratch role
                        # ---- QKV projections ----
                        qt, kt_ = [], []
                        for h in range(H):
                            tq = qkp.tile([DH, L], BF, tag=f"q{h}")
                            tk = qkp.tile([DH, L], BF, tag=f"k{h}")
                            qt.append(tq)
                            kt_.append(tk)
                        vt = []
                        for tt in range(L // 128):
                            tv = vap.tile([128, 8 * 97], BF, tag=f"v{tt}")
                            vt.append(tv)
                        for c in range(NCH):
                            cs = slice(c * CH, (c + 1) * CH)
                            for h in range(H):
                                hs = slice(h * DH, (h + 1) * DH)
                                pq = pmm.tile([DH, CH], F32, tag="mm")
                                for k in range(KT):
                                    nc.tensor.matmul(pq[:], wqt[k][:, hs],
                                                     X[k][:, cs],
                                                     start=(k == 0),
                                                     stop=(k == KT - 1))
                                nc.vector.tensor_scalar_add(qt[h][:, cs], pq[:],
                                                            bqt[:, h:h + 1])
                                pk = pmm.tile([DH, CH], F32, tag="mm")
                                for k in range(KT):
                                    nc.tensor.matmul(pk[:], wkt[k][:, hs],
                                                     X[k][:, cs],
                                                     start=(k == 0),
                                                     stop=(k == KT - 1))
                                nc.vector.tensor_scalar_add(kt_[h][:, cs], pk[:],
                                                            bkt[:, h:h + 1])
                            for tt in range(CH // 128):
                                tg = c * (CH // 128) + tt
                                tok = slice(tg * 128, (tg + 1) * 128)
                                for hg in range(2):
                                    pv = pmm.tile([128, 4 * DH], F32, tag="mm")
                                    for k in range(KT):
                                        nc.tensor.matmul(
                                            pv[:], X[k][:, tok],
                                            wvt[k][:, hg * 4 * DH:(hg + 1) * 4 * DH],
                                            start=(k == 0), stop=(k == KT - 1))
                                    for hh in range(4):
                                        h = hg * 4 + hh
                                        nc.vector.scalar_tensor_tensor(
                                            vt[tg][:, h * 97:h * 97 + DH],
                                            pv[:, hh * DH:(hh + 1) * DH], 1.0,
                                            bvbt[:, h * 97:h * 97 + DH],
                                            op0=AL.mult, op1=AL.add)
                                nc.vector.tensor_copy(vt[tg][:, 96:8 * 97:97],
                                                      bvbt[:, 96:8 * 97:97])
                        # ---- attention + O-proj, both chunks ----
                        wor = []
                        for h in range(H):
                            twh = wstr.tile([DH, D], BF, tag="wo", bufs=8,
                                            name=f"wo{h}")
                            nc.sync.dma_start(twh[:],
                                              wo[i, h * DH:(h + 1) * DH, :])
                            wor.append(twh)
                        for c in range(NCH):
                            cs = slice(c * CH, (c + 1) * CH)
                            ktc = 4 * (c + 1)
                            ot = []
                            for h in range(H):
                                pts = []
                                for kt2 in range(ktc):
                                    ks2 = slice(kt2 * 128, (kt2 + 1) * 128)
                                    psc = pmm.tile([128, CH], F32, tag="mm")
                                    nc.tensor.matmul(psc[:], kt_[h][:, ks2],
                                                     qt[h][:, cs],
                                                     start=True, stop=True)
                                    ptile = ptp.tile([128, CH], BF, tag="pt")
                                    rt = kt2 - 4 * c
                                    if rt >= 0:
                                        tmp = scr.tile([128, CH], F32, tag="scr")
                                        nc.vector.scalar_tensor_tensor(
                                            tmp[:], psc[:], 1.0, maskt[rt],
                                            op0=AL.mult, op1=AL.add)
                                        nc.scalar.activation(ptile[:], tmp[:], AF.Exp)
                                    else:
                                        nc.scalar.activation(ptile[:], psc[:], AF.Exp)
                                    pts.append(ptile)
                                po = ppv.tile([DH + 1, CH], F32, tag="pv")
                                for kt2 in range(ktc):
                                    nc.tensor.matmul(
                                        po[:], vt[kt2][:, h * 97:h * 97 + 97],
                                        pts[kt2][:],
                                        start=(kt2 == 0), stop=(kt2 == ktc - 1))
                                dinv = abp.tile([1, CH], BF, tag="ab", name="dinv")
                                nc.vector.reciprocal(dinv[:], po[DH:DH + 1, :])
                                dib = abp.tile([DH, CH], BF, tag="abb")
                                nc.gpsimd.partition_broadcast(dib[:], dinv[:])
                                oht = otp.tile([DH, CH], BF, tag=f"o{h}",
                                               name=f"o{h}")
                                nc.vector.scalar_tensor_tensor(
                                    oht[:], po[0:DH, :], 1.0, dib[:],
                                    op0=AL.mult, op1=AL.mult)
                                ot.append(oht)
                            for m in range(MT):
                                ms = slice(m * 128, (m + 1) * 128)
                                pp = pmm.tile([128, CH], F32, tag="mm")
                                for h in range(H):
                                    nc.tensor.matmul(pp[:], wor[h][:, ms], ot[h][:],
                                                     start=(h == 0),
                                                     stop=(h == H - 1))
                                nc.vector.scalar_tensor_tensor(
                                    R[m][:, cs], pp[:], bot[:, m:m + 1],
                                    X[m][:, cs], op0=AL.add, op1=AL.add)
                        # LN1 (beta fused with cross-attn bias) -> X role
                        for c in range(NCH):
                            ln_chunk(b, c, R, X, l1g, l1n, cabt[b])
                        # LN2 -> R role
                        for c in range(NCH):
                            ln_chunk(b, c, X, R, lp["l2g"], lp["l2n"], lp["l2b"])
                        # ---- FFN on R -> X role, both chunks ----
                        for c in range(NCH):
                            cs = slice(c * CH, (c + 1) * CH)
                            ht = [hp.tile([128, CH], BF, tag=f"h{m}", name=f"h{m}")
                                  for m in range(FFT)]
                            for mg in range(FFT // 2):
                                colg = slice(mg * 256, (mg + 1) * 256)
                                w1g = []
                                for k in range(KT):
                                    t = wstr.tile([128, 256], BF, tag="w1",
                                                  bufs=12, name=f"w1_{k}")
                                    nc.sync.dma_start(t[:], w1[i, k * 128:(k + 1) * 128, colg])
                                    w1g.append(t)
                                for mi in range(2):
                                    m = mg * 2 + mi
                                    p1 = pmm.tile([128, CH], F32, tag="mm")
                                    for k in range(KT):
                                        nc.tensor.matmul(
                                            p1[:], w1g[k][:, mi * 128:(mi + 1) * 128],
                                            R[k][:, cs],
                                            start=(k == 0), stop=(k == KT - 1))
                                    nc.scalar.activation(ht[m][:], p1[:], AF.Relu,
                                                         bias=b1t[:, m:m + 1])
                            for grp in range(2):
                                p2s = [pmm.tile([128, CH], F32, tag="mm",
                                                name=f"p2_{mi}")
                                       for mi in range(3)]
                                for k in range(FFT):
                                    t = w2str.tile([128, 3 * 128], BF, tag="w2", bufs=6)
                                    nc.sync.dma_start(
                                        t[:], w2[i, k * 128:(k + 1) * 128,
                                                 grp * 384:(grp + 1) * 384])
                                    for mi in range(3):
                                        nc.tensor.matmul(
                                            p2s[mi][:], t[:, mi * 128:(mi + 1) * 128],
                                            ht[k][:],
                                            start=(k == 0), stop=(k == FFT - 1))
                                for mi in range(3):
                                    m = grp * 3 + mi
                                    nc.vector.scalar_tensor_tensor(
                                        X[m][:, cs], p2s[mi][:], b2t[:, m:m + 1],
                                        R[m][:, cs], op0=AL.add, op1=AL.add)
                        # LN3 -> R role
                        for c in range(NCH):
                            ln_chunk(b, c, X, R, lp["l3g"], lp["l3n"], lp["l3b"])
                        cur[b] = 1 - cur[b]


                # ---------- action head ----------
                for b in range(CPC):
                    X = xt[b][cur[b]]
                    for c in range(NCH):
                        cs = slice(c * CH, (c + 1) * CH)
                        pf = pmm.tile([A_DIM, CH], F32, tag="mm")
                        for k in range(KT):
                            nc.tensor.matmul(pf[:], fct[k][:], X[k][:, cs],
                                             start=(k == 0), stop=(k == KT - 1))
                        yt = scr.tile([A_DIM, CH], F32, tag="scr")
                        nc.vector.tensor_scalar_add(yt[:], pf[:], fcbt[:])
                        nc.sync.dma_start(y[b, :, cs], yt[:])


            for _rep in range(reps):
                emit_forward()

    nc.compile()
    return nc


def _posenc(length, d):
    pos_ = np.arange(length, dtype=np.float32)[:, None]
    i = np.arange(0, d, 2, dtype=np.float32)[None, :]
    ang = pos_ / np.power(np.float32(10000.0), i / np.float32(d))
    pe = np.zeros((length, d), np.float32)
    pe[:, 0::2] = np.sin(ang)
    pe[:, 1::2] = np.cos(ang)
    return pe


def _host_prep(inp):
    f32 = np.float32
    a, r, s, t = (np.asarray(inp[k]) for k in ("a", "r", "s", "t"))
    ars = np.concatenate(
        [np.asarray(a, f32), np.asarray(r, f32), np.asarray(s, f32)],
        axis=-1).transpose(0, 2, 1)  # [B, 193, L]
    ars = np.ascontiguousarray(ars).astype(bf)

    scale = f32(1.0 / np.sqrt(DH))
    sa_Wqkv = np.asarray(inp["sa_Wqkv"], f32)
    sa_bqkv = np.asarray(inp["sa_bqkv"], f32)
    wq = (sa_Wqkv[:, 0] * scale).astype(bf)
    wk = sa_Wqkv[:, 1].astype(bf)
    wv = sa_Wqkv[:, 2].astype(bf)
    bq = sa_bqkv[:, 0] * scale
    bk = sa_bqkv[:, 1]
    bv = sa_bqkv[:, 2]
    bvb = np.zeros((NB, 128, 8 * 97), f32)
    for h in range(H):
        bvb[:, :, h * 97:h * 97 + DH] = bv[:, None, h * DH:(h + 1) * DH]
        bvb[:, :, h * 97 + DH] = 1.0
    pcol = np.arange(128)[:, None]
    ucol = np.arange(896)[None, :]
    masks = np.where(pcol > ucol - 384, f32(-30000.0), f32(0.0))

    task_table = np.asarray(inp["task_table"], f32)
    ca_Wqkv = np.asarray(inp["ca_Wqkv"], f32)
    ca_bqkv = np.asarray(inp["ca_bqkv"], f32)
    ca_Wo = np.asarray(inp["ca_Wo"], f32)
    ca_bo = np.asarray(inp["ca_bo"], f32)
    ln1_b = np.asarray(inp["ln1_b"], f32)
    enc = task_table[np.asarray(t)[:, 0]]  # [B, D]
    cab = np.zeros((NB, B, D), f32)
    for i in range(NB):
        v_ = enc @ ca_Wqkv[i, 2] + ca_bqkv[i, 2]
        cab[i] = v_ @ ca_Wo[i] + ca_bo[i]
    cabb_all = cab + ln1_b[:, None, :]  # [NB, B, D]

    ln1_g = np.asarray(inp["ln1_g"], f32)
    ln1gs = np.stack([ln1_g, -ln1_g], axis=1)  # [NB, 2, D]
    lnp_arr = np.stack([
        np.stack([np.asarray(inp["ln2_g"], f32), -np.asarray(inp["ln2_g"], f32),
                  np.asarray(inp["ln2_b"], f32)], axis=1),
        np.stack([np.asarray(inp["ln3_g"], f32), -np.asarray(inp["ln3_g"], f32),
                  np.asarray(inp["ln3_b"], f32)], axis=1),
    ], axis=1)  # [NB, 2, 3, D]
    ln_g = np.asarray(inp["ln_g"], f32)
    lnp0_arr = np.stack([ln_g, -ln_g, np.asarray(inp["ln_b"], f32)])

    shared = dict(
        wa=np.asarray(inp["Wa"], f32).astype(bf),
        wr=np.asarray(inp["Wr"], f32).astype(bf),
        ws=np.asarray(inp["Ws"], f32).astype(bf),
        bemb=np.concatenate([np.asarray(inp["ba"], f32),
                             np.asarray(inp["br"], f32),
                             np.asarray(inp["bs"], f32)]),
        lnp0=lnp0_arr,
        pos=np.ascontiguousarray(_posenc(L, D).T),
        wq=wq, wk=wk, wv=wv,
        wo=np.asarray(inp["sa_Wo"], f32).astype(bf),
        w1=np.asarray(inp["ff_W1"], f32).astype(bf),
        w2=np.asarray(inp["ff_W2"], f32).astype(bf),
        bq=bq, bk=bk, bvb=bvb.astype(bf),
        bo=np.asarray(inp["sa_bo"], f32),
        b1=np.asarray(inp["ff_b1"], f32),
        b2=np.asarray(inp["ff_b2"], f32),
        ln1g=ln1gs, lnp=lnp_arr,
        masks=masks.astype(bf),
        fcw=np.asarray(inp["fc_W"], f32).astype(bf),
        fcb=np.asarray(inp["fc_b"], f32),
    )
    in_maps = []
    for core in range(NCORES):
        m = dict(shared)
        m["ars"] = ars[core * CPC:(core + 1) * CPC]
        m["cabb"] = np.ascontiguousarray(
            cabb_all[:, core * CPC:(core + 1) * CPC])
        in_maps.append(m)
    return in_maps


def _get_nc(reps=1):
    key = f"nc{reps}"
    if key not in _CACHE:
        _CACHE[key] = _build(reps)
    return _CACHE[key]


def kernel(**inputs):
    nc = _get_nc()
    in_maps = _host_prep(inputs)
    res = run_bass_kernel_spmd(nc, in_maps, core_ids=list(range(NCORES)))
    out = np.zeros((B, L, A_DIM), np.float32)
    for core in range(NCORES):
        yc = res.results[core]["y"]  # [CPC, 64, L]
        for b in range(CPC):
            out[core * CPC + b] = yc[b].T
    return out



# revision 19
# speedup vs baseline: 1.5619x; 1.5619x over previous
"""Trainium2 Bass kernel for nn_DiscreteDecisionTransformer.

Decision-transformer forward: embed(a,r,s) -> LN -> +posenc, then 4 blocks of
[causal self-attn, cross-attn, FFN] with post-LN, then action head.

Distribution: data-parallel over batch, 16 batches / 8 cores = 2 per core.
Params replicated; zero collectives. Feature-major activations ([dmodel on
partitions, tokens on free dim]) so GEMMs contract over partitions.

v2 changes vs baseline:
 - Two-batch software pipeline: batch A's attention (exp/softmax on Act/DVE)
   is interleaved at head/micro-group granularity with batch B's FFN (PE
   heavy), so the PE never waits on softmax or LayerNorm chains.
 - All weights host-packed partition-major and DMA'd in large merged
   transfers (~200 DMAs/core vs ~2200): HWDGE descriptor-generation was a
   serialized 1.4ms in the baseline.
 - Per-block scalar params packed into one [128,112] f32 tensor (1 DMA).

Host prep (unchanged math): cross-attn collapses to a per-(block,batch) bias
fused into LN1 beta; 1/sqrt(dh) folded into Wq; additive causal mask tiles;
softmax denominators via ones-column appended to V (97-col heads); LN stats
(sum, sumsq) on the PE with a ones-vector lhsT.

GEMMs in bf16 with f32 PSUM accumulation.
"""

import sys
from contextlib import ExitStack

sys.path.insert(0, "/opt/trn_rl_repo")

import numpy as np
import ml_dtypes

import concourse.bacc as bacc
import concourse.mybir as mybir
import concourse.tile as tile
from concourse.bass_utils import run_bass_kernel_spmd

bf = ml_dtypes.bfloat16

B, L, D, H, DH, NB, E = 16, 1024, 768, 8, 96, 4, 256
A_DIM, S_DIM = 64, 128
NCORES = 8
CPC = B // NCORES  # batches per core
KT = D // 128      # 6 k-tiles of dmodel
MT = D // 128      # 6 m-tiles of dmodel
CH = 512           # token chunk (matmul N)
NCH = L // CH      # 2 chunks per batch
FFT = 4 * D // 128 # 24 m-tiles of ffn hidden
F32, BF = mybir.dt.float32, mybir.dt.bfloat16
AL = mybir.AluOpType
AF = mybir.ActivationFunctionType

# biasblk column layout
_BQ, _BK, _BO, _B1, _B2, _CAB, _L1G, _L1N = 0, 8, 16, 22, 46, 52, 64, 70
_L2G, _L2N, _L2B, _L3G, _L3N, _L3B = 76, 82, 88, 94, 100, 106

_CACHE = {}


def _build(reps=1):
    """Emit the full per-core program. Returns the finished Bacc object."""
    nc = bacc.Bacc("TRN2", target_bir_lowering=False, debug=False)
    dram = nc.dram_tensor

    ars = dram("ars", [CPC, 193, L], BF, kind="ExternalInput")
    wab = dram("wab", [65, E], BF, kind="ExternalInput")
    wsd = dram("wsd", [S_DIM, E], BF, kind="ExternalInput")
    emb0 = dram("emb0", [128, 24], F32, kind="ExternalInput")
    pos = dram("pos", [D, L], F32, kind="ExternalInput")
    wq = dram("wq", [NB, 128, KT * D], BF, kind="ExternalInput")
    wk = dram("wk", [NB, 128, KT * D], BF, kind="ExternalInput")
    wv = dram("wv", [NB, 128, KT * D], BF, kind="ExternalInput")
    wo = dram("wo", [NB, DH, H * D], BF, kind="ExternalInput")
    w1 = dram("w1", [NB, 128, KT, 4 * D], BF, kind="ExternalInput")
    w2 = dram("w2", [NB, 128, FFT, D], BF, kind="ExternalInput")
    bvb = dram("bvb", [NB, 128, 8 * 97], BF, kind="ExternalInput")
    bblk = dram("bblk", [NB, 128, 112], F32, kind="ExternalInput")
    masks = dram("masks", [128, 896], BF, kind="ExternalInput")
    fcw = dram("fcw", [128, KT * A_DIM], BF, kind="ExternalInput")
    fcb = dram("fcb", [A_DIM], F32, kind="ExternalInput")
    y = dram("y", [CPC, A_DIM, L], F32, kind="ExternalOutput")

    with nc.allow_low_precision(reason="bf16 kernel by design"), \
         tile.TileContext(nc) as tc, ExitStack() as ctx:
            ep = ctx.enter_context
            cst = ep(tc.tile_pool(name="cst", bufs=1))
            wqp = ep(tc.tile_pool(name="wqp", bufs=1))
            wsp = ep(tc.tile_pool(name="wsp", bufs=3))
            bsp = ep(tc.tile_pool(name="bsp", bufs=2))
            xp = ep(tc.tile_pool(name="xp", bufs=1))
            qkp = ep(tc.tile_pool(name="qk", bufs=1))
            vap = ep(tc.tile_pool(name="vap", bufs=1))
            ptp = ep(tc.tile_pool(name="ptp", bufs=8))
            otp = ep(tc.tile_pool(name="otp", bufs=1))
            scr = ep(tc.tile_pool(name="scr", bufs=2))
            hp = ep(tc.tile_pool(name="hp", bufs=1))
            smv = ep(tc.tile_pool(name="smv", bufs=2))
            abp = ep(tc.tile_pool(name="abp", bufs=2))
            pmm = ep(tc.tile_pool(name="pmm", bufs=5, space="PSUM"))
            ppv = ep(tc.tile_pool(name="ppv", bufs=2, space="PSUM"))
            pst = ep(tc.tile_pool(name="pst", bufs=1, space="PSUM"))
            # ---------- global constants ----------
            ones = cst.tile([128, 1], BF)
            nc.any.memset(ones[:], 1.0)
            epst = cst.tile([1, 1], F32)
            nc.any.memset(epst[:], 1e-5)
            bigm = cst.tile([128, 896], BF, tag="bigm")
            nc.sync.dma_start(bigm[:], masks[:])
            maskt = [bigm[:, 384 - rt * 128:896 - rt * 128] for rt in range(4)]
            fct = cst.tile([128, KT * A_DIM], BF, tag="fcw")
            nc.sync.dma_start(fct[:], fcw[:])
            fcbt = cst.tile([A_DIM, 1], F32, tag="fcb")
            nc.sync.dma_start(fcbt[:], fcb[:].rearrange("(m o) -> m o", o=1))
            wabt = cst.tile([65, E], BF, tag="wab")
            nc.sync.dma_start(wabt[:], wab[:])
            wst = cst.tile([S_DIM, E], BF, tag="ws")
            nc.sync.dma_start(wst[:], wsd[:])
            emb0t = cst.tile([128, 24], F32, tag="emb0")
            nc.sync.dma_start(emb0t[:], emb0[:])

            # residual-stream tiles, two roles that alternate per LN
            xt = [[[xp.tile([128, L], BF, tag=f"x{b}_{j}_{k}", name=f"x{b}_{j}_{k}")
                    for k in range(KT)]
                   for j in range(2)] for b in range(CPC)]
            # attention working set (single set shared by both batches)
            ht = [hp.tile([128, CH], BF, tag=f"h{m}", name=f"h{m}")
                  for m in range(FFT)]

            def ln_chunk(IN, OUT, gt, gnt, bt_, c, post_pos=False):
                """LayerNorm over features for one 512-token chunk."""
                cs = slice(c * CH, (c + 1) * CH)
                st = pst.tile([33, CH], F32, tag="st")
                for k in range(KT):
                    nc.tensor.matmul(st[0:1, :], ones[:], IN[k][:, cs],
                                     start=(k == 0), stop=(k == KT - 1))
                for k in range(KT):
                    xsq = scr.tile([128, CH], BF, tag="xsq", bufs=3)
                    nc.scalar.activation(xsq[:], IN[k][:, cs], AF.Square)
                    nc.tensor.matmul(st[32:33, :], ones[:], xsq[:],
                                     start=(k == 0), stop=(k == KT - 1))
                mu = smv.tile([1, CH], F32, tag="mu", bufs=1)
                nc.vector.tensor_scalar_mul(mu[:], st[0:1, :], 1.0 / D)
                m2 = smv.tile([1, CH], F32, tag="sm", bufs=3)
                nc.vector.tensor_scalar_mul(m2[:], st[32:33, :], 1.0 / D)
                mu2 = smv.tile([1, CH], F32, tag="sm", bufs=3)
                nc.vector.tensor_mul(mu2[:], mu[:], mu[:])
                var = smv.tile([1, CH], F32, tag="sm", bufs=3)
                nc.vector.tensor_sub(var[:], m2[:], mu2[:])
                sd = smv.tile([1, CH], F32, tag="sm", bufs=3)
                nc.scalar.activation(sd[:], var[:], AF.Sqrt, bias=epst[:])
                ab = abp.tile([1, 2 * CH], BF, tag="ab", bufs=1)
                nc.vector.reciprocal(ab[:, 0:CH], sd[:])
                nc.vector.tensor_mul(ab[:, CH:2 * CH], mu[:], ab[:, 0:CH])
                abb = abp.tile([128, 2 * CH], BF, tag="abb", bufs=1)
                nc.gpsimd.partition_broadcast(abb[:], ab[:])
                for k in range(KT):
                    u = scr.tile([128, CH], F32, tag="scr")
                    nc.vector.scalar_tensor_tensor(
                        u[:], IN[k][:, cs], gt[:, k:k + 1], abb[:, 0:CH],
                        op0=AL.mult, op1=AL.mult)
                    w_ = scr.tile([128, CH], F32, tag="scr")
                    nc.vector.scalar_tensor_tensor(
                        w_[:], abb[:, CH:2 * CH], gnt[:, k:k + 1], u[:],
                        op0=AL.mult, op1=AL.add)
                    if post_pos:
                        t2 = scr.tile([128, CH], F32, tag="scr")
                        nc.scalar.activation(t2[:], w_[:], AF.Identity,
                                             bias=bt_[:, k:k + 1])
                        pe = scr.tile([128, CH], F32, tag="scr")
                        nc.sync.dma_start(pe[:], pos[k * 128:(k + 1) * 128, cs])
                        nc.vector.tensor_add(OUT[k][:, cs], t2[:], pe[:])
                    else:
                        nc.scalar.activation(OUT[k][:, cs], w_[:], AF.Identity,
                                             bias=bt_[:, k:k + 1])

            def emit_forward():
                # ---------- embed + LN + posenc ----------
                def embed_chunk(b, c):
                    cs = slice(c * CH, (c + 1) * CH)
                    ta = scr.tile([65, CH], BF, tag="xsq", bufs=3)
                    nc.sync.dma_start(ta[:], ars[b, 0:65, cs])
                    ts = scr.tile([S_DIM, CH], BF, tag="xsq", bufs=3)
                    nc.sync.dma_start(ts[:], ars[b, 65:193, cs])
                    for m in range(MT):
                        p = pmm.tile([128, CH], F32, tag="mm")
                        ms = slice((m % 2) * 128, (m % 2) * 128 + 128)
                        if m < 2:
                            nc.tensor.matmul(p[:], wabt[0:64, ms], ta[0:64, :],
                                             start=True, stop=True)
                        elif m < 4:
                            nc.tensor.matmul(p[:], wabt[64:65, ms], ta[64:65, :],
                                             start=True, stop=True)
                        else:
                            nc.tensor.matmul(p[:], wst[:, ms], ts[:],
                                             start=True, stop=True)
                        nc.scalar.activation(xt[b][0][m][:, cs], p[:],
                                             AF.Identity,
                                             bias=emb0t[:, m:m + 1])
                    ln_chunk(xt[b][0], xt[b][1], emb0t[:, 6:12],
                             emb0t[:, 12:18], emb0t[:, 18:24], c,
                             post_pos=True)

                # roles: after embed, x lives in role 1
                cur = [1, 1]

                # ---- per-block weight loads; handles kept per block ----
                def wload(i):
                    ws_ = {}
                    for nm, src in (("wq", wq), ("wk", wk), ("wv", wv)):
                        t = wqp.tile([128, KT * D], BF, tag=nm, name=f"{nm}{i}")
                        nc.sync.dma_start(t[:], src[i])
                        ws_[nm] = t
                    return ws_

                def wload2(i, ws_):
                    t = wqp.tile([DH, H * D], BF, tag="wo", name=f"wo{i}")
                    nc.sync.dma_start(t[:], wo[i])
                    ws_["wo"] = t
                    bt = bsp.tile([128, 112], F32, tag="bblk", name=f"bblk{i}")
                    nc.sync.dma_start(bt[:], bblk[i])
                    ws_["bb"] = bt
                    bv = bsp.tile([128, 8 * 97], BF, tag="bvb", name=f"bvb{i}", bufs=1)
                    nc.sync.dma_start(bv[:], bvb[i])
                    ws_["bvb"] = bv

                # attention working-set tiles (allocated fresh per (batch,block))
                def alloc_qkv():
                    qt = [qkp.tile([DH, L], BF, tag=f"q{h}", name=f"q{h}")
                          for h in range(H)]
                    kt_ = [qkp.tile([DH, L], BF, tag=f"k{h}", name=f"k{h}")
                           for h in range(H)]
                    vt = [vap.tile([128, 8 * 97], BF, tag=f"v{tt}", name=f"v{tt}")
                          for tt in range(L // 128)]
                    return qt, kt_, vt

                def qkv_chunk(b, c, ws_, qkvt):
                    """Q/K/V projections for one 512-token chunk."""
                    X = xt[b][cur[b]]
                    qt, kt_, vt = qkvt
                    wqt, wkt, wvt = ws_["wq"], ws_["wk"], ws_["wv"]
                    bb = ws_["bb"]
                    bvbt = ws_["bvb"]
                    cs = slice(c * CH, (c + 1) * CH)
                    for h in range(H):
                        pq = pmm.tile([DH, CH], F32, tag="mm")
                        for k in range(KT):
                            nc.tensor.matmul(
                                pq[:], wqt[:, k * D + h * DH:k * D + (h + 1) * DH],
                                X[k][:, cs], start=(k == 0), stop=(k == KT - 1))
                        nc.vector.tensor_scalar_add(qt[h][:, cs], pq[:],
                                                    bb[0:DH, _BQ + h:_BQ + h + 1])
                        pk = pmm.tile([DH, CH], F32, tag="mm")
                        for k in range(KT):
                            nc.tensor.matmul(
                                pk[:], wkt[:, k * D + h * DH:k * D + (h + 1) * DH],
                                X[k][:, cs], start=(k == 0), stop=(k == KT - 1))
                        nc.vector.tensor_scalar_add(kt_[h][:, cs], pk[:],
                                                    bb[0:DH, _BK + h:_BK + h + 1])
                    for tt in range(CH // 128):
                        tg = c * (CH // 128) + tt
                        tok = slice(tg * 128, (tg + 1) * 128)
                        for hg in range(2):
                            pv = pmm.tile([128, 4 * DH], F32, tag="mm")
                            for k in range(KT):
                                nc.tensor.matmul(
                                    pv[:], X[k][:, tok],
                                    wvt[:, k * D + hg * 4 * DH:k * D + (hg + 1) * 4 * DH],
                                    start=(k == 0), stop=(k == KT - 1))
                            for hh in range(4):
                                h = hg * 4 + hh
                                nc.vector.scalar_tensor_tensor(
                                    vt[tg][:, h * 97:h * 97 + DH],
                                    pv[:, hh * DH:(hh + 1) * DH], 1.0,
                                    bvbt[:, h * 97:h * 97 + DH],
                                    op0=AL.mult, op1=AL.add)
                        nc.vector.tensor_copy(vt[tg][:, 96:8 * 97:97],
                                              bvbt[:, 96:8 * 97:97])

                def att_chunk(b, c, qkvt, ot, zips):
                    """Scores+softmax+PV for one chunk; `zips` is a list of
                    closures (other batch's FFN micro-groups) interleaved after
                    each head's scores so the PE never waits on exp."""
                    qt, kt_, vt = qkvt
                    cs = slice(c * CH, (c + 1) * CH)
                    ktc = 4 * (c + 1)
                    zi = iter(zips)
                    for h in range(H):
                        pts = []
                        for kt2 in range(ktc):
                            ks2 = slice(kt2 * 128, (kt2 + 1) * 128)
                            psc = pmm.tile([128, CH], F32, tag="mm")
                            nc.tensor.matmul(psc[:], kt_[h][:, ks2],
                                             qt[h][:, cs], start=True, stop=True)
                            ptile = ptp.tile([128, CH], BF, tag="pt")
                            rt = kt2 - 4 * c
                            if rt >= 0:
                                tmp = scr.tile([128, CH], F32, tag="scr")
                                nc.vector.scalar_tensor_tensor(
                                    tmp[:], psc[:], 1.0, maskt[rt],
                                    op0=AL.mult, op1=AL.add)
                                nc.scalar.activation(ptile[:], tmp[:], AF.Exp)
                            else:
                                nc.scalar.activation(ptile[:], psc[:], AF.Exp)
                            pts.append(ptile)
                        for z in (next(zi, None), next(zi, None),
                                  next(zi, None)):
                            if z is not None:
                                z()
                        po = ppv.tile([DH + 1, CH], F32, tag="pv")
                        for kt2 in range(ktc):
                            nc.tensor.matmul(
                                po[:], vt[kt2][:, h * 97:h * 97 + 97], pts[kt2][:],
                                start=(kt2 == 0), stop=(kt2 == ktc - 1))
                        dinv = abp.tile([1, CH], BF, tag="dinv", name="dinv", bufs=1)
                        nc.vector.reciprocal(dinv[:], po[DH:DH + 1, :])
                        dib = abp.tile([DH, CH], BF, tag="dib", name="dib", bufs=1)
                        nc.gpsimd.partition_broadcast(dib[:], dinv[:])
                        oht = otp.tile([DH, CH], BF, tag=f"o{h}", name=f"o{h}")
                        nc.vector.scalar_tensor_tensor(
                            oht[:], po[0:DH, :], 1.0, dib[:],
                            op0=AL.mult, op1=AL.mult)
                        ot[h] = oht
                    for z in zi:
                        z()

                def o_chunk(b, c, ws_, ot):
                    X = xt[b][cur[b]]
                    R = xt[b][1 - cur[b]]
                    wot = ws_["wo"]
                    bb = ws_["bb"]
                    cs = slice(c * CH, (c + 1) * CH)
                    for m in range(MT):
                        ms = slice(m * 128, (m + 1) * 128)
                        pp = pmm.tile([128, CH], F32, tag="mm")
                        for h in range(H):
                            nc.tensor.matmul(pp[:],
                                             wot[:, h * D + m * 128:h * D + (m + 1) * 128],
                                             ot[h][:], start=(h == 0),
                                             stop=(h == H - 1))
                        nc.vector.scalar_tensor_tensor(
                            R[m][:, cs], pp[:], bb[:, _BO + m:_BO + m + 1],
                            X[m][:, cs], op0=AL.add, op1=AL.add)

                def ffn_micros(b, c, i):
                    """FFN for one chunk as 16 closures (8 ffn1 + 8 ffn2)."""
                    X = xt[b][cur[b]]
                    R = xt[b][1 - cur[b]]

                    def f1(e, ws_):
                        def run():
                            wt = wsp.tile([128, KT * 256], BF, tag="wst",
                                          name=f"w1_{e}")
                            nc.sync.dma_start(wt[:],
                                              w1[i, :, :, e * 256:(e + 1) * 256])
                            for mi in range(2):
                                m = 2 * e + mi
                                p1 = pmm.tile([128, CH], F32, tag="mm")
                                for k in range(KT):
                                    nc.tensor.matmul(
                                        p1[:],
                                        wt[:, k * 256 + mi * 128:k * 256 + (mi + 1) * 128],
                                        R[k][:, c * CH:(c + 1) * CH],
                                        start=(k == 0), stop=(k == KT - 1))
                                nc.scalar.activation(
                                    ht[m][:], p1[:], AF.Relu,
                                    bias=ws_["bb"][:, _B1 + m:_B1 + m + 1])
                        return run

                    p2s = {}

                    def f2(g, kq, ws_):
                        def run():
                            wt = wsp.tile([128, 4 * 384], BF, tag="wst",
                                          name=f"w2_{g}_{kq}")
                            nc.sync.dma_start(
                                wt[:], w2[i, :, kq * 4:(kq + 1) * 4,
                                           g * 384:(g + 1) * 384])
                            if kq == 0:
                                p2s[g] = [pmm.tile([128, CH], F32, tag="mm",
                                                   name=f"p2_{g}_{mi}")
                                          for mi in range(3)]
                            for kk in range(4):
                                k = kq * 4 + kk
                                for mi in range(3):
                                    nc.tensor.matmul(
                                        p2s[g][mi][:],
                                        wt[:, kk * 384 + mi * 128:kk * 384 + (mi + 1) * 128],
                                        ht[k][:], start=(k == 0),
                                        stop=(k == FFT - 1))
                            if kq == 5:
                                cs = slice(c * CH, (c + 1) * CH)
                                for mi in range(3):
                                    m = g * 3 + mi
                                    nc.vector.scalar_tensor_tensor(
                                        X[m][:, cs], p2s[g][mi][:],
                                        ws_["bb"][:, _B2 + m:_B2 + m + 1],
                                        R[m][:, cs], op0=AL.add, op1=AL.add)
                        return run
                    return f1, f2

                def ffn_micro_list(b, c, i, ws_):
                    f1, f2 = ffn_micros(b, c, i)
                    return ([f1(e, ws_) for e in range(12)] +
                            [f2(g, kq, ws_) for g in range(2) for kq in range(6)])

                def ln_stage(b, which, ws_, c):
                    bb = ws_["bb"]
                    X = xt[b][cur[b]]
                    R = xt[b][1 - cur[b]]
                    if which == 1:
                        ln_chunk(R, X, bb[:, _L1G:_L1G + 6], bb[:, _L1N:_L1N + 6],
                                 bb[:, _CAB + b * 6:_CAB + (b + 1) * 6], c)
                    elif which == 2:
                        ln_chunk(X, R, bb[:, _L2G:_L2G + 6], bb[:, _L2N:_L2N + 6],
                                 bb[:, _L2B:_L2B + 6], c)
                    else:
                        ln_chunk(X, R, bb[:, _L3G:_L3G + 6], bb[:, _L3N:_L3N + 6],
                                 bb[:, _L3B:_L3B + 6], c)

                def head_chunk(b, c):
                    Xf = xt[b][cur[b]]
                    cs = slice(c * CH, (c + 1) * CH)
                    pf = pmm.tile([A_DIM, CH], F32, tag="mm")
                    for k in range(KT):
                        nc.tensor.matmul(pf[:], fct[:, k * A_DIM:(k + 1) * A_DIM],
                                         Xf[k][:, cs], start=(k == 0),
                                         stop=(k == KT - 1))
                    yt = scr.tile([A_DIM, CH], F32, tag="scr")
                    nc.vector.tensor_scalar_add(yt[:], pf[:], fcbt[:])
                    nc.sync.dma_start(y[b, :, cs], yt[:])

                # ---------- transformer blocks: 2-batch pipeline ----------
                WS = wload(0)
                wload2(0, WS)
                WSn = None
                qkv_b = [None, None]   # live qkv tiles per batch
                # prologue: embeds, then b0's QKV (overlaps embed LN tails)
                embed_chunk(0, 0)
                embed_chunk(0, 1)
                embed_chunk(1, 0)
                embed_chunk(1, 1)
                qkv_b[0] = alloc_qkv()
                qkv_chunk(0, 0, WS, qkv_b[0])
                qkv_chunk(0, 1, WS, qkv_b[0])
                ot_b = [[None] * H, [None] * H]
                pend = None            # (b, ws, micros c0, micros c1) FFN of
                                       # other batch from previous block
                for i in range(NB):
                    for b in (0, 1):
                        o = 1 - b
                        if b == 0 and i > 0:
                            WS = WSn
                        if qkv_b[b] is None:  # prologue only (b0, block 0)
                            qkv_b[b] = alloc_qkv()
                            for c in range(NCH):
                                qkv_chunk(b, c, WS, qkv_b[b])
                        # attention zipped with other batch's pending FFN
                        if pend is not None:
                            pb, pws, pm0, pm1 = pend
                            att_chunk(b, 0, qkv_b[b], ot_b[b], pm0)
                            o_chunk(b, 0, WS, ot_b[b])
                            att_chunk(b, 1, qkv_b[b], ot_b[b], pm1)
                            ln_stage(pb, 3, pws, 0)
                            o_chunk(b, 1, WS, ot_b[b])
                            ln_stage(pb, 3, pws, 1)
                            cur[pb] = 1 - cur[pb]
                        else:
                            att_chunk(b, 0, qkv_b[b], ot_b[b], [])
                            o_chunk(b, 0, WS, ot_b[b])
                            att_chunk(b, 1, qkv_b[b], ot_b[b], [])
                            o_chunk(b, 1, WS, ot_b[b])
                        # LN1 zipped with other batch's QKV (next user of the
                        # single qkv tile set)
                        ln_stage(b, 1, WS, 0)
                        if b == 0:
                            # b1's qkv for this block
                            qkv_b[o] = alloc_qkv()
                            qkv_chunk(o, 0, WS, qkv_b[o])
                            ln_stage(b, 1, WS, 1)
                            qkv_chunk(o, 1, WS, qkv_b[o])
                            WSn = wload(i + 1) if i + 1 < NB else None
                        else:
                            if i + 1 < NB:
                                # b0's qkv for next block (uses next weights)
                                wload2(i + 1, WSn)
                                qkv_b[o] = alloc_qkv()
                                qkv_chunk(o, 0, WSn, qkv_b[o])
                                ln_stage(b, 1, WS, 1)
                                qkv_chunk(o, 1, WSn, qkv_b[o])
                            else:
                                ln_stage(b, 1, WS, 1)
                                qkv_b[o] = None
                        ln_stage(b, 2, WS, 0)
                        ln_stage(b, 2, WS, 1)
                        pend = (b, WS, ffn_micro_list(b, 0, i, WS),
                                ffn_micro_list(b, 1, i, WS))

                # ---------- epilogue: b1's last FFN + LN3, then head ----------
                pb, pws, pm0, pm1 = pend
                for z in pm0:
                    z()
                head_chunk(0, 0)
                for z in pm1:
                    z()
                head_chunk(0, 1)
                ln_stage(pb, 3, pws, 0)
                ln_stage(pb, 3, pws, 1)
                cur[1] = 1 - cur[1]
                head_chunk(1, 0)
                head_chunk(1, 1)

            for _rep in range(reps):
                emit_forward()

    nc.compile()
    return nc


def _posenc(length, d):
    pos_ = np.arange(length, dtype=np.float32)[:, None]
    i = np.arange(0, d, 2, dtype=np.float32)[None, :]
    ang = pos_ / np.power(np.float32(10000.0), i / np.float32(d))
    pe = np.zeros((length, d), np.float32)
    pe[:, 0::2] = np.sin(ang)
    pe[:, 1::2] = np.cos(ang)
    return pe


def _pack_pk(w, p=128):
    """[K*p, M] -> [p, K*M] partition-major packing."""
    k = w.shape[0] // p
    return np.ascontiguousarray(
        w.reshape(k, p, w.shape[1]).transpose(1, 0, 2).reshape(p, -1))


def _host_prep(inp):
    f32 = np.float32
    a, r, s, t = (np.asarray(inp[k]) for k in ("a", "r", "s", "t"))
    ars = np.concatenate(
        [np.asarray(a, f32), np.asarray(r, f32), np.asarray(s, f32)],
        axis=-1).transpose(0, 2, 1)  # [B, 193, L]
    ars = np.ascontiguousarray(ars).astype(bf)

    scale = f32(1.0 / np.sqrt(DH))
    sa_Wqkv = np.asarray(inp["sa_Wqkv"], f32)
    sa_bqkv = np.asarray(inp["sa_bqkv"], f32)
    wq_p = np.stack([_pack_pk((sa_Wqkv[i, 0] * scale).astype(bf))
                     for i in range(NB)])
    wk_p = np.stack([_pack_pk(sa_Wqkv[i, 1].astype(bf)) for i in range(NB)])
    wv_p = np.stack([_pack_pk(sa_Wqkv[i, 2].astype(bf)) for i in range(NB)])
    wo_p = np.stack([_pack_pk(np.asarray(inp["sa_Wo"], f32)[i].astype(bf), p=DH)
                     for i in range(NB)])
    w1_p = np.stack([
        _pack_pk(np.asarray(inp["ff_W1"], f32)[i].astype(bf)).reshape(
            128, KT, 4 * D) for i in range(NB)])
    w2_p = np.stack([
        _pack_pk(np.asarray(inp["ff_W2"], f32)[i].astype(bf)).reshape(
            128, FFT, D) for i in range(NB)])

    bq = sa_bqkv[:, 0] * scale
    bk = sa_bqkv[:, 1]
    bv = sa_bqkv[:, 2]
    bvb = np.zeros((NB, 128, 8 * 97), f32)
    for h in range(H):
        bvb[:, :, h * 97:h * 97 + DH] = bv[:, None, h * DH:(h + 1) * DH]
        bvb[:, :, h * 97 + DH] = 1.0
    pcol = np.arange(128)[:, None]
    ucol = np.arange(896)[None, :]
    masks = np.where(pcol > ucol - 384, f32(-30000.0), f32(0.0))

    task_table = np.asarray(inp["task_table"], f32)
    ca_Wqkv = np.asarray(inp["ca_Wqkv"], f32)
    ca_bqkv = np.asarray(inp["ca_bqkv"], f32)
    ca_Wo = np.asarray(inp["ca_Wo"], f32)
    ca_bo = np.asarray(inp["ca_bo"], f32)
    ln1_b = np.asarray(inp["ln1_b"], f32)
    enc = task_table[np.asarray(t)[:, 0]]  # [B, D]
    cab = np.zeros((NB, B, D), f32)
    for i in range(NB):
        v_ = enc @ ca_Wqkv[i, 2] + ca_bqkv[i, 2]
        cab[i] = v_ @ ca_Wo[i] + ca_bo[i]
    cabb_all = cab + ln1_b[:, None, :]  # [NB, B, D]

    def cols(x, p=128):
        # [n*p] -> [p, n] column pack
        return x.reshape(-1, p).T

    ln1_g = np.asarray(inp["ln1_g"], f32)
    l2g = np.asarray(inp["ln2_g"], f32)
    l2b = np.asarray(inp["ln2_b"], f32)
    l3g = np.asarray(inp["ln3_g"], f32)
    l3b = np.asarray(inp["ln3_b"], f32)
    bo_ = np.asarray(inp["sa_bo"], f32)
    b1_ = np.asarray(inp["ff_b1"], f32)
    b2_ = np.asarray(inp["ff_b2"], f32)

    bblk_shared = np.zeros((NB, 128, 112), f32)
    for i in range(NB):
        bblk_shared[i, 0:DH, _BQ:_BQ + 8] = bq[i].reshape(H, DH).T
        bblk_shared[i, 0:DH, _BK:_BK + 8] = bk[i].reshape(H, DH).T
        bblk_shared[i, :, _BO:_BO + 6] = cols(bo_[i])
        bblk_shared[i, :, _B1:_B1 + 24] = cols(b1_[i])
        bblk_shared[i, :, _B2:_B2 + 6] = cols(b2_[i])
        bblk_shared[i, :, _L1G:_L1G + 6] = cols(ln1_g[i])
        bblk_shared[i, :, _L1N:_L1N + 6] = cols(-ln1_g[i])
        bblk_shared[i, :, _L2G:_L2G + 6] = cols(l2g[i])
        bblk_shared[i, :, _L2N:_L2N + 6] = cols(-l2g[i])
        bblk_shared[i, :, _L2B:_L2B + 6] = cols(l2b[i])
        bblk_shared[i, :, _L3G:_L3G + 6] = cols(l3g[i])
        bblk_shared[i, :, _L3N:_L3N + 6] = cols(-l3g[i])
        bblk_shared[i, :, _L3B:_L3B + 6] = cols(l3b[i])

    ln_g = np.asarray(inp["ln_g"], f32)
    ln_b = np.asarray(inp["ln_b"], f32)
    bemb = np.concatenate([np.asarray(inp["ba"], f32),
                           np.asarray(inp["br"], f32),
                           np.asarray(inp["bs"], f32)])
    emb0 = np.zeros((128, 24), f32)
    emb0[:, 0:6] = cols(bemb)
    emb0[:, 6:12] = cols(ln_g)
    emb0[:, 12:18] = cols(-ln_g)
    emb0[:, 18:24] = cols(ln_b)

    wab = np.concatenate([np.asarray(inp["Wa"], f32),
                          np.asarray(inp["Wr"], f32)], axis=0)  # [65, E]
    fcw_p = _pack_pk(np.asarray(inp["fc_W"], f32).astype(bf))  # [128, 6*64]

    shared = dict(
        wab=wab.astype(bf),
        wsd=np.asarray(inp["Ws"], f32).astype(bf),
        emb0=emb0,
        pos=np.ascontiguousarray(_posenc(L, D).T),
        wq=wq_p, wk=wk_p, wv=wv_p, wo=wo_p, w1=w1_p, w2=w2_p,
        bvb=bvb.astype(bf),
        masks=masks.astype(bf),
        fcw=fcw_p,
        fcb=np.asarray(inp["fc_b"], f32),
    )
    in_maps = []
    for core in range(NCORES):
        m = dict(shared)
        m["ars"] = ars[core * CPC:(core + 1) * CPC]
        bb = bblk_shared.copy()
        for i in range(NB):
            for b in range(CPC):
                bb[i, :, _CAB + b * 6:_CAB + (b + 1) * 6] = cols(
                    cabb_all[i, core * CPC + b])
        m["bblk"] = bb
        in_maps.append(m)
    return in_maps


def _get_nc(reps=1):
    key = f"nc{reps}"
    if key not in _CACHE:
        _CACHE[key] = _build(reps)
    return _CACHE[key]


def kernel(**inputs):
    nc = _get_nc()
    in_maps = _host_prep(inputs)
    res = run_bass_kernel_spmd(nc, in_maps, core_ids=list(range(NCORES)))
    out = np.zeros((B, L, A_DIM), np.float32)
    for core in range(NCORES):
        yc = res.results[core]["y"]  # [CPC, 64, L]
        for b in range(CPC):
            out[core * CPC + b] = yc[b].T
    return out


# revision 22
# speedup vs baseline: 1.6385x; 1.0491x over previous
"""Trainium2 Bass kernel for nn_DiscreteDecisionTransformer.

Decision-transformer forward: embed(a,r,s) -> LN -> +posenc, then 4 blocks of
[causal self-attn, cross-attn, FFN] with post-LN, then action head.

Distribution: data-parallel over batch, 16 batches / 8 cores = 2 per core.
Params replicated; zero collectives. Feature-major activations ([dmodel on
partitions, tokens on free dim]) so GEMMs contract over partitions.

v2 changes vs baseline:
 - Two-batch software pipeline: batch A's attention (exp/softmax on Act/DVE)
   is interleaved at head/micro-group granularity with batch B's FFN (PE
   heavy), so the PE never waits on softmax or LayerNorm chains.
 - All weights host-packed partition-major and DMA'd in large merged
   transfers (~200 DMAs/core vs ~2200): HWDGE descriptor-generation was a
   serialized 1.4ms in the baseline.
 - Per-block scalar params packed into one [128,112] f32 tensor (1 DMA).

Host prep (unchanged math): cross-attn collapses to a per-(block,batch) bias
fused into LN1 beta; 1/sqrt(dh) folded into Wq; additive causal mask tiles;
softmax denominators via ones-column appended to V (97-col heads); LN stats
(sum, sumsq) on the PE with a ones-vector lhsT.

GEMMs in bf16 with f32 PSUM accumulation.
"""

import sys
from contextlib import ExitStack

sys.path.insert(0, "/opt/trn_rl_repo")

import numpy as np
import ml_dtypes

import concourse.bacc as bacc
import concourse.mybir as mybir
import concourse.tile as tile
from concourse.bass_utils import run_bass_kernel_spmd

bf = ml_dtypes.bfloat16

B, L, D, H, DH, NB, E = 16, 1024, 768, 8, 96, 4, 256
A_DIM, S_DIM = 64, 128
NCORES = 8
CPC = B // NCORES  # batches per core
KT = D // 128      # 6 k-tiles of dmodel
MT = D // 128      # 6 m-tiles of dmodel
CH = 512           # token chunk (matmul N)
NCH = L // CH      # 2 chunks per batch
FFT = 4 * D // 128 # 24 m-tiles of ffn hidden
F32, BF = mybir.dt.float32, mybir.dt.bfloat16
AL = mybir.AluOpType
AF = mybir.ActivationFunctionType

# biasblk column layout
_BQ, _BK, _BO, _B1, _B2, _CAB, _L1G, _L1N = 0, 8, 16, 22, 46, 52, 64, 70
_L2G, _L2N, _L2B, _L3G, _L3N, _L3B = 76, 82, 88, 94, 100, 106

_CACHE = {}


def _build(reps=1):
    """Emit the full per-core program. Returns the finished Bacc object."""
    nc = bacc.Bacc("TRN2", target_bir_lowering=False, debug=False)
    dram = nc.dram_tensor

    ars = dram("ars", [CPC, 193, L], BF, kind="ExternalInput")
    wab = dram("wab", [65, E], BF, kind="ExternalInput")
    wsd = dram("wsd", [S_DIM, E], BF, kind="ExternalInput")
    emb0 = dram("emb0", [128, 24], F32, kind="ExternalInput")
    pos = dram("pos", [D, L], F32, kind="ExternalInput")
    wq = dram("wq", [NB, 128, KT * D], BF, kind="ExternalInput")
    wk = dram("wk", [NB, 128, KT * D], BF, kind="ExternalInput")
    wv = dram("wv", [NB, 128, KT * D], BF, kind="ExternalInput")
    wo = dram("wo", [NB, DH, H * D], BF, kind="ExternalInput")
    w1 = dram("w1", [NB, 128, FFT, KT * 128], BF, kind="ExternalInput")
    w2 = dram("w2", [NB, 128, FFT, D], BF, kind="ExternalInput")
    bvb = dram("bvb", [NB, 128, 8 * 97], BF, kind="ExternalInput")
    bblk = dram("bblk", [NB, 128, 112], F32, kind="ExternalInput")
    masks = dram("masks", [128, 896], BF, kind="ExternalInput")
    fcw = dram("fcw", [128, KT * A_DIM], BF, kind="ExternalInput")
    fcb = dram("fcb", [A_DIM], F32, kind="ExternalInput")
    y = dram("y", [CPC, A_DIM, L], F32, kind="ExternalOutput")

    with nc.allow_low_precision(reason="bf16 kernel by design"), \
         tile.TileContext(nc) as tc, ExitStack() as ctx:
            ep = ctx.enter_context
            cst = ep(tc.tile_pool(name="cst", bufs=1))
            wqp = ep(tc.tile_pool(name="wqp", bufs=1))
            wsp = ep(tc.tile_pool(name="wsp", bufs=6))
            bsp = ep(tc.tile_pool(name="bsp", bufs=2))
            xp = ep(tc.tile_pool(name="xp", bufs=1))
            qkp = ep(tc.tile_pool(name="qk", bufs=1))
            vap = ep(tc.tile_pool(name="vap", bufs=1))
            ptp = ep(tc.tile_pool(name="ptp", bufs=8))
            otp = ep(tc.tile_pool(name="otp", bufs=1))
            scr = ep(tc.tile_pool(name="scr", bufs=2))
            hp = ep(tc.tile_pool(name="hp", bufs=1))
            smv = ep(tc.tile_pool(name="smv", bufs=2))
            abp = ep(tc.tile_pool(name="abp", bufs=2))
            pmm = ep(tc.tile_pool(name="pmm", bufs=5, space="PSUM"))
            ppv = ep(tc.tile_pool(name="ppv", bufs=2, space="PSUM"))
            pst = ep(tc.tile_pool(name="pst", bufs=1, space="PSUM"))
            # ---------- global constants ----------
            ones = cst.tile([128, 1], BF)
            nc.any.memset(ones[:], 1.0)
            epst = cst.tile([1, 1], F32)
            nc.any.memset(epst[:], 1e-5)
            bigm = cst.tile([128, 896], BF, tag="bigm")
            nc.sync.dma_start(bigm[:], masks[:])
            maskt = [bigm[:, 384 - rt * 128:896 - rt * 128] for rt in range(4)]
            fct = cst.tile([128, KT * A_DIM], BF, tag="fcw")
            nc.sync.dma_start(fct[:], fcw[:])
            fcbt = cst.tile([A_DIM, 1], F32, tag="fcb")
            nc.sync.dma_start(fcbt[:], fcb[:].rearrange("(m o) -> m o", o=1))
            wabt = cst.tile([65, E], BF, tag="wab")
            nc.sync.dma_start(wabt[:], wab[:])
            wst = cst.tile([S_DIM, E], BF, tag="ws")
            nc.sync.dma_start(wst[:], wsd[:])
            emb0t = cst.tile([128, 24], F32, tag="emb0")
            nc.sync.dma_start(emb0t[:], emb0[:])

            # residual-stream tiles, two roles that alternate per LN
            xt = [[[xp.tile([128, L], BF, tag=f"x{b}_{j}_{k}", name=f"x{b}_{j}_{k}")
                    for k in range(KT)]
                   for j in range(2)] for b in range(CPC)]
            # attention working set (single set shared by both batches)
            ht = [hp.tile([128, CH], BF, tag=f"h{m}", name=f"h{m}")
                  for m in range(FFT)]

            def ln_chunk(IN, OUT, gt, gnt, bt_, c, post_pos=False):
                """LayerNorm over features for one 512-token chunk."""
                cs = slice(c * CH, (c + 1) * CH)
                st = pst.tile([33, CH], F32, tag="st")
                for k in range(KT):
                    nc.tensor.matmul(st[0:1, :], ones[:], IN[k][:, cs],
                                     start=(k == 0), stop=(k == KT - 1))
                for k in range(KT):
                    xsq = scr.tile([128, CH], BF, tag="xsq", bufs=3)
                    nc.scalar.activation(xsq[:], IN[k][:, cs], AF.Square)
                    nc.tensor.matmul(st[32:33, :], ones[:], xsq[:],
                                     start=(k == 0), stop=(k == KT - 1))
                mu = smv.tile([1, CH], F32, tag="mu", bufs=1)
                nc.vector.tensor_scalar_mul(mu[:], st[0:1, :], 1.0 / D)
                m2 = smv.tile([1, CH], F32, tag="sm", bufs=3)
                nc.vector.tensor_scalar_mul(m2[:], st[32:33, :], 1.0 / D)
                mu2 = smv.tile([1, CH], F32, tag="sm", bufs=3)
                nc.vector.tensor_mul(mu2[:], mu[:], mu[:])
                var = smv.tile([1, CH], F32, tag="sm", bufs=3)
                nc.vector.tensor_sub(var[:], m2[:], mu2[:])
                sd = smv.tile([1, CH], F32, tag="sm", bufs=3)
                nc.scalar.activation(sd[:], var[:], AF.Sqrt, bias=epst[:])
                ab = abp.tile([1, 2 * CH], BF, tag="ab", bufs=1)
                nc.vector.reciprocal(ab[:, 0:CH], sd[:])
                nc.vector.tensor_mul(ab[:, CH:2 * CH], mu[:], ab[:, 0:CH])
                abb = abp.tile([128, 2 * CH], BF, tag="abb", bufs=1)
                nc.gpsimd.partition_broadcast(abb[:], ab[:])
                for k in range(KT):
                    u = scr.tile([128, CH], F32, tag="scr")
                    nc.vector.scalar_tensor_tensor(
                        u[:], IN[k][:, cs], gt[:, k:k + 1], abb[:, 0:CH],
                        op0=AL.mult, op1=AL.mult)
                    w_ = scr.tile([128, CH], F32, tag="scr")
                    nc.vector.scalar_tensor_tensor(
                        w_[:], abb[:, CH:2 * CH], gnt[:, k:k + 1], u[:],
                        op0=AL.mult, op1=AL.add)
                    if post_pos:
                        pe = scr.tile([128, CH], F32, tag="scr")
                        nc.sync.dma_start(pe[:], pos[k * 128:(k + 1) * 128, cs])
                        nc.vector.tensor_add(OUT[k][:, cs], w_[:], pe[:])
                    else:
                        nc.scalar.activation(OUT[k][:, cs], w_[:], AF.Identity,
                                             bias=bt_[:, k:k + 1])

            def emit_forward():
                # ---------- embed + LN + posenc ----------
                def embed_chunk(b, c):
                    cs = slice(c * CH, (c + 1) * CH)
                    ta = scr.tile([65, CH], BF, tag="xsq", bufs=3)
                    nc.sync.dma_start(ta[:], ars[b, 0:65, cs])
                    ts = scr.tile([S_DIM, CH], BF, tag="xsq", bufs=3)
                    nc.sync.dma_start(ts[:], ars[b, 65:193, cs])
                    for m in range(MT):
                        p = pmm.tile([128, CH], F32, tag="mm")
                        ms = slice((m % 2) * 128, (m % 2) * 128 + 128)
                        if m < 2:
                            nc.tensor.matmul(p[:], wabt[0:64, ms], ta[0:64, :],
                                             start=True, stop=True)
                        elif m < 4:
                            nc.tensor.matmul(p[:], wabt[64:65, ms], ta[64:65, :],
                                             start=True, stop=True)
                        else:
                            nc.tensor.matmul(p[:], wst[:, ms], ts[:],
                                             start=True, stop=True)
                        nc.scalar.activation(xt[b][0][m][:, cs], p[:],
                                             AF.Identity,
                                             bias=emb0t[:, m:m + 1])
                    ln_chunk(xt[b][0], xt[b][1], emb0t[:, 6:12],
                             emb0t[:, 12:18], emb0t[:, 18:24], c,
                             post_pos=True)

                # roles: after embed, x lives in role 1
                cur = [1, 1]

                # ---- per-block weight loads; handles kept per block ----
                def wload(i):
                    ws_ = {}
                    for nm, src in (("wq", wq), ("wk", wk), ("wv", wv)):
                        t = wqp.tile([128, KT * D], BF, tag=nm, name=f"{nm}{i}")
                        nc.sync.dma_start(t[:], src[i])
                        ws_[nm] = t
                    return ws_

                def wload2(i, ws_):
                    t = wqp.tile([DH, H * D], BF, tag="wo", name=f"wo{i}")
                    nc.sync.dma_start(t[:], wo[i])
                    ws_["wo"] = t
                    bt = bsp.tile([128, 112], F32, tag="bblk", name=f"bblk{i}")
                    nc.sync.dma_start(bt[:], bblk[i])
                    ws_["bb"] = bt
                    bv = bsp.tile([128, 8 * 97], BF, tag="bvb", name=f"bvb{i}", bufs=1)
                    nc.sync.dma_start(bv[:], bvb[i])
                    ws_["bvb"] = bv

                # attention working-set tiles (allocated fresh per (batch,block))
                def alloc_qkv():
                    qt = [qkp.tile([DH, L], BF, tag=f"q{h}", name=f"q{h}")
                          for h in range(H)]
                    kt_ = [qkp.tile([DH, L], BF, tag=f"k{h}", name=f"k{h}")
                           for h in range(H)]
                    vt = [vap.tile([128, 8 * 97], BF, tag=f"v{tt}", name=f"v{tt}")
                          for tt in range(L // 128)]
                    return qt, kt_, vt

                def qkv_chunk(b, c, ws_, qkvt):
                    """Q/K/V projections for one 512-token chunk."""
                    X = xt[b][cur[b]]
                    qt, kt_, vt = qkvt
                    wqt, wkt, wvt = ws_["wq"], ws_["wk"], ws_["wv"]
                    bb = ws_["bb"]
                    bvbt = ws_["bvb"]
                    cs = slice(c * CH, (c + 1) * CH)
                    for h in range(H):
                        pq = pmm.tile([DH, CH], F32, tag="mm")
                        for k in range(KT):
                            nc.tensor.matmul(
                                pq[:], wqt[:, k * D + h * DH:k * D + (h + 1) * DH],
                                X[k][:, cs], start=(k == 0), stop=(k == KT - 1))
                        nc.vector.tensor_scalar_add(qt[h][:, cs], pq[:],
                                                    bb[0:DH, _BQ + h:_BQ + h + 1])
                        pk = pmm.tile([DH, CH], F32, tag="mm")
                        for k in range(KT):
                            nc.tensor.matmul(
                                pk[:], wkt[:, k * D + h * DH:k * D + (h + 1) * DH],
                                X[k][:, cs], start=(k == 0), stop=(k == KT - 1))
                        nc.vector.tensor_scalar_add(kt_[h][:, cs], pk[:],
                                                    bb[0:DH, _BK + h:_BK + h + 1])
                    for tt in range(CH // 128):
                        tg = c * (CH // 128) + tt
                        tok = slice(tg * 128, (tg + 1) * 128)
                        for hg in range(2):
                            pv = pmm.tile([128, 4 * DH], F32, tag="mm")
                            for k in range(KT):
                                nc.tensor.matmul(
                                    pv[:], X[k][:, tok],
                                    wvt[:, k * D + hg * 4 * DH:k * D + (hg + 1) * 4 * DH],
                                    start=(k == 0), stop=(k == KT - 1))
                            for hh in range(4):
                                h = hg * 4 + hh
                                nc.vector.scalar_tensor_tensor(
                                    vt[tg][:, h * 97:h * 97 + DH],
                                    pv[:, hh * DH:(hh + 1) * DH], 1.0,
                                    bvbt[:, h * 97:h * 97 + DH],
                                    op0=AL.mult, op1=AL.add)
                        nc.vector.tensor_copy(vt[tg][:, 96:8 * 97:97],
                                              bvbt[:, 96:8 * 97:97])

                def att_chunk(b, c, qkvt, ot, zips):
                    """Scores+softmax+PV for one chunk; `zips` is a list of
                    closures (other batch's FFN micro-groups) interleaved after
                    each head's scores so the PE never waits on exp."""
                    qt, kt_, vt = qkvt
                    cs = slice(c * CH, (c + 1) * CH)
                    ktc = 4 * (c + 1)
                    zi = iter(zips)
                    for h in range(H):
                        pts = []
                        for kt2 in range(ktc):
                            ks2 = slice(kt2 * 128, (kt2 + 1) * 128)
                            psc = pmm.tile([128, CH], F32, tag="mm")
                            nc.tensor.matmul(psc[:], kt_[h][:, ks2],
                                             qt[h][:, cs], start=True, stop=True)
                            ptile = ptp.tile([128, CH], BF, tag="pt")
                            rt = kt2 - 4 * c
                            if rt >= 0:
                                tmp = scr.tile([128, CH], F32, tag="scr")
                                nc.vector.scalar_tensor_tensor(
                                    tmp[:], psc[:], 1.0, maskt[rt],
                                    op0=AL.mult, op1=AL.add)
                                nc.scalar.activation(ptile[:], tmp[:], AF.Exp)
                            else:
                                nc.scalar.activation(ptile[:], psc[:], AF.Exp)
                            pts.append(ptile)
                        for _ in range(6):
                            z = next(zi, None)
                            if z is not None:
                                z()
                        po = ppv.tile([DH + 1, CH], F32, tag="pv")
                        for kt2 in range(ktc):
                            nc.tensor.matmul(
                                po[:], vt[kt2][:, h * 97:h * 97 + 97], pts[kt2][:],
                                start=(kt2 == 0), stop=(kt2 == ktc - 1))
                        dinv = abp.tile([1, CH], BF, tag="dinv", name="dinv", bufs=1)
                        nc.vector.reciprocal(dinv[:], po[DH:DH + 1, :])
                        dib = abp.tile([DH, CH], BF, tag="dib", name="dib", bufs=1)
                        nc.gpsimd.partition_broadcast(dib[:], dinv[:])
                        oht = otp.tile([DH, CH], BF, tag=f"o{h}", name=f"o{h}")
                        nc.vector.scalar_tensor_tensor(
                            oht[:], po[0:DH, :], 1.0, dib[:],
                            op0=AL.mult, op1=AL.mult)
                        ot[h] = oht
                    for z in zi:
                        z()

                def o_chunk(b, c, ws_, ot):
                    X = xt[b][cur[b]]
                    R = xt[b][1 - cur[b]]
                    wot = ws_["wo"]
                    bb = ws_["bb"]
                    cs = slice(c * CH, (c + 1) * CH)
                    for m in range(MT):
                        ms = slice(m * 128, (m + 1) * 128)
                        pp = pmm.tile([128, CH], F32, tag="mm")
                        for h in range(H):
                            nc.tensor.matmul(pp[:],
                                             wot[:, h * D + m * 128:h * D + (m + 1) * 128],
                                             ot[h][:], start=(h == 0),
                                             stop=(h == H - 1))
                        nc.vector.scalar_tensor_tensor(
                            R[m][:, cs], pp[:], bb[:, _BO + m:_BO + m + 1],
                            X[m][:, cs], op0=AL.add, op1=AL.add)

                def ffn_micros(b, c, i):
                    """FFN for one chunk as 16 closures (8 ffn1 + 8 ffn2)."""
                    X = xt[b][cur[b]]
                    R = xt[b][1 - cur[b]]

                    def f1(m, ws_):
                        def run():
                            wt = wsp.tile([128, KT * 128], BF, tag="wst",
                                          name=f"w1_{m}")
                            nc.sync.dma_start(wt[:], w1[i, :, m, :])
                            p1 = pmm.tile([128, CH], F32, tag="mm")
                            for k in range(KT):
                                nc.tensor.matmul(
                                    p1[:], wt[:, k * 128:(k + 1) * 128],
                                    R[k][:, c * CH:(c + 1) * CH],
                                    start=(k == 0), stop=(k == KT - 1))
                            nc.scalar.activation(
                                ht[m][:], p1[:], AF.Relu,
                                bias=ws_["bb"][:, _B1 + m:_B1 + m + 1])
                        return run

                    p2s = {}

                    def f2(g, kq, ws_):
                        def run():
                            wt = wsp.tile([128, 2 * 384], BF, tag="wst",
                                          name=f"w2_{g}_{kq}")
                            nc.sync.dma_start(
                                wt[:], w2[i, :, kq * 2:(kq + 1) * 2,
                                           g * 384:(g + 1) * 384])
                            if kq == 0:
                                p2s[g] = [pmm.tile([128, CH], F32, tag="mm",
                                                   name=f"p2_{g}_{mi}")
                                          for mi in range(3)]
                            for kk in range(2):
                                k = kq * 2 + kk
                                for mi in range(3):
                                    nc.tensor.matmul(
                                        p2s[g][mi][:],
                                        wt[:, kk * 384 + mi * 128:kk * 384 + (mi + 1) * 128],
                                        ht[k][:], start=(k == 0),
                                        stop=(k == FFT - 1))
                            if kq == 11:
                                cs = slice(c * CH, (c + 1) * CH)
                                for mi in range(3):
                                    m = g * 3 + mi
                                    nc.vector.scalar_tensor_tensor(
                                        X[m][:, cs], p2s[g][mi][:],
                                        ws_["bb"][:, _B2 + m:_B2 + m + 1],
                                        R[m][:, cs], op0=AL.add, op1=AL.add)
                        return run
                    return f1, f2

                def ffn_micro_list(b, c, i, ws_):
                    f1, f2 = ffn_micros(b, c, i)
                    return ([f1(m, ws_) for m in range(FFT)] +
                            [f2(g, kq, ws_) for g in range(2) for kq in range(12)])

                def ln_stage(b, which, ws_, c):
                    bb = ws_["bb"]
                    X = xt[b][cur[b]]
                    R = xt[b][1 - cur[b]]
                    if which == 1:
                        ln_chunk(R, X, bb[:, _L1G:_L1G + 6], bb[:, _L1N:_L1N + 6],
                                 bb[:, _CAB + b * 6:_CAB + (b + 1) * 6], c)
                    elif which == 2:
                        ln_chunk(X, R, bb[:, _L2G:_L2G + 6], bb[:, _L2N:_L2N + 6],
                                 bb[:, _L2B:_L2B + 6], c)
                    else:
                        ln_chunk(X, R, bb[:, _L3G:_L3G + 6], bb[:, _L3N:_L3N + 6],
                                 bb[:, _L3B:_L3B + 6], c)

                def head_chunk(b, c):
                    Xf = xt[b][cur[b]]
                    cs = slice(c * CH, (c + 1) * CH)
                    pf = pmm.tile([A_DIM, CH], F32, tag="mm")
                    for k in range(KT):
                        nc.tensor.matmul(pf[:], fct[:, k * A_DIM:(k + 1) * A_DIM],
                                         Xf[k][:, cs], start=(k == 0),
                                         stop=(k == KT - 1))
                    yt = scr.tile([A_DIM, CH], F32, tag="scr")
                    nc.vector.tensor_scalar_add(yt[:], pf[:], fcbt[:])
                    nc.sync.dma_start(y[b, :, cs], yt[:])

                # ---------- transformer blocks: 2-batch pipeline ----------
                WS = wload(0)
                wload2(0, WS)
                WSn = None
                qkv_b = [None, None]   # live qkv tiles per batch
                # prologue: embeds, then b0's QKV (overlaps embed LN tails)
                embed_chunk(0, 0)
                embed_chunk(0, 1)
                embed_chunk(1, 0)
                embed_chunk(1, 1)
                qkv_b[0] = alloc_qkv()
                qkv_chunk(0, 0, WS, qkv_b[0])
                qkv_chunk(0, 1, WS, qkv_b[0])
                ot_b = [[None] * H, [None] * H]
                pend = None            # (b, ws, micros c0, micros c1) FFN of
                                       # other batch from previous block
                for i in range(NB):
                    for b in (0, 1):
                        o = 1 - b
                        if b == 0 and i > 0:
                            WS = WSn
                        if qkv_b[b] is None:  # prologue only (b0, block 0)
                            qkv_b[b] = alloc_qkv()
                            for c in range(NCH):
                                qkv_chunk(b, c, WS, qkv_b[b])
                        # attention zipped with other batch's pending FFN
                        if pend is not None:
                            pb, pws, pm0, pm1 = pend
                            att_chunk(b, 0, qkv_b[b], ot_b[b], pm0)
                            o_chunk(b, 0, WS, ot_b[b])
                            att_chunk(b, 1, qkv_b[b], ot_b[b], pm1)
                            ln_stage(pb, 3, pws, 0)
                            o_chunk(b, 1, WS, ot_b[b])
                            ln_stage(pb, 3, pws, 1)
                            cur[pb] = 1 - cur[pb]
                        else:
                            att_chunk(b, 0, qkv_b[b], ot_b[b], [])
                            o_chunk(b, 0, WS, ot_b[b])
                            att_chunk(b, 1, qkv_b[b], ot_b[b], [])
                            o_chunk(b, 1, WS, ot_b[b])
                        # LN1 zipped with other batch's QKV (next user of the
                        # single qkv tile set)
                        ln_stage(b, 1, WS, 0)
                        if b == 0:
                            # b1's qkv for this block
                            qkv_b[o] = alloc_qkv()
                            qkv_chunk(o, 0, WS, qkv_b[o])
                            ln_stage(b, 1, WS, 1)
                            qkv_chunk(o, 1, WS, qkv_b[o])
                            WSn = wload(i + 1) if i + 1 < NB else None
                        else:
                            if i + 1 < NB:
                                # b0's qkv for next block (uses next weights)
                                wload2(i + 1, WSn)
                                qkv_b[o] = alloc_qkv()
                                qkv_chunk(o, 0, WSn, qkv_b[o])
                                ln_stage(b, 1, WS, 1)
                                qkv_chunk(o, 1, WSn, qkv_b[o])
                            else:
                                ln_stage(b, 1, WS, 1)
                                qkv_b[o] = None
                        ln_stage(b, 2, WS, 0)
                        ln_stage(b, 2, WS, 1)
                        pend = (b, WS, ffn_micro_list(b, 0, i, WS),
                                ffn_micro_list(b, 1, i, WS))

                # ---------- epilogue: b1's last FFN + LN3, then head ----------
                pb, pws, pm0, pm1 = pend
                for z in pm0:
                    z()
                head_chunk(0, 0)
                for z in pm1:
                    z()
                head_chunk(0, 1)
                ln_stage(pb, 3, pws, 0)
                ln_stage(pb, 3, pws, 1)
                cur[1] = 1 - cur[1]
                head_chunk(1, 0)
                head_chunk(1, 1)

            for _rep in range(reps):
                emit_forward()

    nc.compile()
    return nc


def _posenc(length, d):
    pos_ = np.arange(length, dtype=np.float32)[:, None]
    i = np.arange(0, d, 2, dtype=np.float32)[None, :]
    ang = pos_ / np.power(np.float32(10000.0), i / np.float32(d))
    pe = np.zeros((length, d), np.float32)
    pe[:, 0::2] = np.sin(ang)
    pe[:, 1::2] = np.cos(ang)
    return pe


def _pack_pk(w, p=128):
    """[K*p, M] -> [p, K*M] partition-major packing."""
    k = w.shape[0] // p
    return np.ascontiguousarray(
        w.reshape(k, p, w.shape[1]).transpose(1, 0, 2).reshape(p, -1))


def _host_prep(inp):
    f32 = np.float32
    a, r, s, t = (np.asarray(inp[k]) for k in ("a", "r", "s", "t"))
    ars = np.concatenate(
        [np.asarray(a, f32), np.asarray(r, f32), np.asarray(s, f32)],
        axis=-1).transpose(0, 2, 1)  # [B, 193, L]
    ars = np.ascontiguousarray(ars).astype(bf)

    scale = f32(1.0 / np.sqrt(DH))
    sa_Wqkv = np.asarray(inp["sa_Wqkv"], f32)
    sa_bqkv = np.asarray(inp["sa_bqkv"], f32)
    wq_p = np.stack([_pack_pk((sa_Wqkv[i, 0] * scale).astype(bf))
                     for i in range(NB)])
    wk_p = np.stack([_pack_pk(sa_Wqkv[i, 1].astype(bf)) for i in range(NB)])
    wv_p = np.stack([_pack_pk(sa_Wqkv[i, 2].astype(bf)) for i in range(NB)])
    wo_p = np.stack([_pack_pk(np.asarray(inp["sa_Wo"], f32)[i].astype(bf), p=DH)
                     for i in range(NB)])
    w1_p = np.stack([
        np.ascontiguousarray(
            np.asarray(inp["ff_W1"], f32)[i].astype(bf)
            .reshape(KT, 128, FFT, 128).transpose(1, 2, 0, 3)
            .reshape(128, FFT, KT * 128))
        for i in range(NB)])
    w2_p = np.stack([
        _pack_pk(np.asarray(inp["ff_W2"], f32)[i].astype(bf)).reshape(
            128, FFT, D) for i in range(NB)])

    bq = sa_bqkv[:, 0] * scale
    bk = sa_bqkv[:, 1]
    bv = sa_bqkv[:, 2]
    bvb = np.zeros((NB, 128, 8 * 97), f32)
    for h in range(H):
        bvb[:, :, h * 97:h * 97 + DH] = bv[:, None, h * DH:(h + 1) * DH]
        bvb[:, :, h * 97 + DH] = 1.0
    pcol = np.arange(128)[:, None]
    ucol = np.arange(896)[None, :]
    masks = np.where(pcol > ucol - 384, f32(-30000.0), f32(0.0))

    task_table = np.asarray(inp["task_table"], f32)
    ca_Wqkv = np.asarray(inp["ca_Wqkv"], f32)
    ca_bqkv = np.asarray(inp["ca_bqkv"], f32)
    ca_Wo = np.asarray(inp["ca_Wo"], f32)
    ca_bo = np.asarray(inp["ca_bo"], f32)
    ln1_b = np.asarray(inp["ln1_b"], f32)
    enc = task_table[np.asarray(t)[:, 0]]  # [B, D]
    cab = np.zeros((NB, B, D), f32)
    for i in range(NB):
        v_ = enc @ ca_Wqkv[i, 2] + ca_bqkv[i, 2]
        cab[i] = v_ @ ca_Wo[i] + ca_bo[i]
    cabb_all = cab + ln1_b[:, None, :]  # [NB, B, D]

    def cols(x, p=128):
        # [n*p] -> [p, n] column pack
        return x.reshape(-1, p).T

    ln1_g = np.asarray(inp["ln1_g"], f32)
    l2g = np.asarray(inp["ln2_g"], f32)
    l2b = np.asarray(inp["ln2_b"], f32)
    l3g = np.asarray(inp["ln3_g"], f32)
    l3b = np.asarray(inp["ln3_b"], f32)
    bo_ = np.asarray(inp["sa_bo"], f32)
    b1_ = np.asarray(inp["ff_b1"], f32)
    b2_ = np.asarray(inp["ff_b2"], f32)

    bblk_shared = np.zeros((NB, 128, 112), f32)
    for i in range(NB):
        bblk_shared[i, 0:DH, _BQ:_BQ + 8] = bq[i].reshape(H, DH).T
        bblk_shared[i, 0:DH, _BK:_BK + 8] = bk[i].reshape(H, DH).T
        bblk_shared[i, :, _BO:_BO + 6] = cols(bo_[i])
        bblk_shared[i, :, _B1:_B1 + 24] = cols(b1_[i])
        bblk_shared[i, :, _B2:_B2 + 6] = cols(b2_[i])
        bblk_shared[i, :, _L1G:_L1G + 6] = cols(ln1_g[i])
        bblk_shared[i, :, _L1N:_L1N + 6] = cols(-ln1_g[i])
        bblk_shared[i, :, _L2G:_L2G + 6] = cols(l2g[i])
        bblk_shared[i, :, _L2N:_L2N + 6] = cols(-l2g[i])
        bblk_shared[i, :, _L2B:_L2B + 6] = cols(l2b[i])
        bblk_shared[i, :, _L3G:_L3G + 6] = cols(l3g[i])
        bblk_shared[i, :, _L3N:_L3N + 6] = cols(-l3g[i])
        bblk_shared[i, :, _L3B:_L3B + 6] = cols(l3b[i])

    ln_g = np.asarray(inp["ln_g"], f32)
    ln_b = np.asarray(inp["ln_b"], f32)
    bemb = np.concatenate([np.asarray(inp["ba"], f32),
                           np.asarray(inp["br"], f32),
                           np.asarray(inp["bs"], f32)])
    emb0 = np.zeros((128, 24), f32)
    emb0[:, 0:6] = cols(bemb)
    emb0[:, 6:12] = cols(ln_g)
    emb0[:, 12:18] = cols(-ln_g)
    emb0[:, 18:24] = cols(ln_b)

    wab = np.concatenate([np.asarray(inp["Wa"], f32),
                          np.asarray(inp["Wr"], f32)], axis=0)  # [65, E]
    fcw_p = _pack_pk(np.asarray(inp["fc_W"], f32).astype(bf))  # [128, 6*64]

    shared = dict(
        wab=wab.astype(bf),
        wsd=np.asarray(inp["Ws"], f32).astype(bf),
        emb0=emb0,
        pos=np.ascontiguousarray(_posenc(L, D).T
                                 + ln_b[:, None]),
        wq=wq_p, wk=wk_p, wv=wv_p, wo=wo_p, w1=w1_p, w2=w2_p,
        bvb=bvb.astype(bf),
        masks=masks.astype(bf),
        fcw=fcw_p,
        fcb=np.asarray(inp["fc_b"], f32),
    )
    in_maps = []
    for core in range(NCORES):
        m = dict(shared)
        m["ars"] = ars[core * CPC:(core + 1) * CPC]
        bb = bblk_shared.copy()
        for i in range(NB):
            for b in range(CPC):
                bb[i, :, _CAB + b * 6:_CAB + (b + 1) * 6] = cols(
                    cabb_all[i, core * CPC + b])
        m["bblk"] = bb
        in_maps.append(m)
    return in_maps


def _get_nc(reps=1):
    key = f"nc{reps}"
    if key not in _CACHE:
        _CACHE[key] = _build(reps)
    return _CACHE[key]


def kernel(**inputs):
    nc = _get_nc()
    in_maps = _host_prep(inputs)
    res = run_bass_kernel_spmd(nc, in_maps, core_ids=list(range(NCORES)))
    out = np.zeros((B, L, A_DIM), np.float32)
    for core in range(NCORES):
        yc = res.results[core]["y"]  # [CPC, 64, L]
        for b in range(CPC):
            out[core * CPC + b] = yc[b].T
    return out


# revision 29
# speedup vs baseline: 1.6661x; 1.0168x over previous
"""Trainium2 Bass kernel for nn_DiscreteDecisionTransformer.

Decision-transformer forward: embed(a,r,s) -> LN -> +posenc, then 4 blocks of
[causal self-attn, cross-attn, FFN] with post-LN, then action head.

Distribution: data-parallel over batch, 16 batches / 8 cores = 2 per core.
Params replicated; zero collectives. Feature-major activations ([dmodel on
partitions, tokens on free dim]) so GEMMs contract over partitions.

v2 changes vs baseline:
 - Two-batch software pipeline: batch A's attention (exp/softmax on Act/DVE)
   is interleaved at head/micro-group granularity with batch B's FFN (PE
   heavy), so the PE never waits on softmax or LayerNorm chains.
 - All weights host-packed partition-major and DMA'd in large merged
   transfers (~200 DMAs/core vs ~2200): HWDGE descriptor-generation was a
   serialized 1.4ms in the baseline.
 - Per-block scalar params packed into one [128,112] f32 tensor (1 DMA).

Host prep (unchanged math): cross-attn collapses to a per-(block,batch) bias
fused into LN1 beta; 1/sqrt(dh) folded into Wq; additive causal mask tiles;
softmax denominators via ones-column appended to V (97-col heads); LN stats
(sum, sumsq) on the PE with a ones-vector lhsT.

GEMMs in bf16 with f32 PSUM accumulation.
"""

import sys
from contextlib import ExitStack

sys.path.insert(0, "/opt/trn_rl_repo")

import numpy as np
import ml_dtypes

import concourse.bacc as bacc
import concourse.mybir as mybir
import concourse.tile as tile
from concourse.bass_utils import run_bass_kernel_spmd

bf = ml_dtypes.bfloat16

B, L, D, H, DH, NB, E = 16, 1024, 768, 8, 96, 4, 256
A_DIM, S_DIM = 64, 128
NCORES = 8
CPC = B // NCORES  # batches per core
KT = D // 128      # 6 k-tiles of dmodel
MT = D // 128      # 6 m-tiles of dmodel
CH = 512           # token chunk (matmul N)
NCH = L // CH      # 2 chunks per batch
FFT = 4 * D // 128 # 24 m-tiles of ffn hidden
F32, BF = mybir.dt.float32, mybir.dt.bfloat16
AL = mybir.AluOpType
AF = mybir.ActivationFunctionType

# biasblk column layout
_BQ, _BK, _BO, _B1, _B2, _CAB, _L1G, _L1N = 0, 8, 16, 22, 46, 52, 64, 70
_L2G, _L2N, _L2B, _L3G, _L3N, _L3B = 76, 82, 88, 94, 100, 106

_CACHE = {}


def _build(reps=1):
    """Emit the full per-core program. Returns the finished Bacc object."""
    nc = bacc.Bacc("TRN2", target_bir_lowering=False, debug=False)
    dram = nc.dram_tensor

    ars = dram("ars", [CPC, 193, L], BF, kind="ExternalInput")
    wab = dram("wab", [65, E], BF, kind="ExternalInput")
    wsd = dram("wsd", [S_DIM, E], BF, kind="ExternalInput")
    emb0 = dram("emb0", [128, 24], F32, kind="ExternalInput")
    pos = dram("pos", [D, L], F32, kind="ExternalInput")
    wq = dram("wq", [NB, 128, KT * D], BF, kind="ExternalInput")
    wk = dram("wk", [NB, 128, KT * D], BF, kind="ExternalInput")
    wv = dram("wv", [NB, 128, KT * D], BF, kind="ExternalInput")
    wo = dram("wo", [NB, DH, H * D], BF, kind="ExternalInput")
    w1 = dram("w1", [NB, 128, FFT, KT * 128], BF, kind="ExternalInput")
    w2 = dram("w2", [NB, 128, FFT, D], BF, kind="ExternalInput")
    bvb = dram("bvb", [NB, 128, 8 * 97], BF, kind="ExternalInput")
    bblk = dram("bblk", [NB, 128, 112], F32, kind="ExternalInput")
    masks = dram("masks", [128, 896], BF, kind="ExternalInput")
    fcw = dram("fcw", [128, KT * A_DIM], BF, kind="ExternalInput")
    fcb = dram("fcb", [A_DIM], F32, kind="ExternalInput")
    y = dram("y", [CPC, A_DIM, L], F32, kind="ExternalOutput")

    with nc.allow_low_precision(reason="bf16 kernel by design"), \
         tile.TileContext(nc) as tc, ExitStack() as ctx:
            ep = ctx.enter_context
            cst = ep(tc.tile_pool(name="cst", bufs=1))
            wqp = ep(tc.tile_pool(name="wqp", bufs=1))
            wsp = ep(tc.tile_pool(name="wsp", bufs=6))
            bsp = ep(tc.tile_pool(name="bsp", bufs=2))
            xp = ep(tc.tile_pool(name="xp", bufs=1))
            qkp = ep(tc.tile_pool(name="qk", bufs=1))
            vap = ep(tc.tile_pool(name="vap", bufs=1))
            ptp = ep(tc.tile_pool(name="ptp", bufs=8))
            otp = ep(tc.tile_pool(name="otp", bufs=1))
            scr = ep(tc.tile_pool(name="scr", bufs=2))
            hp = ep(tc.tile_pool(name="hp", bufs=1))
            smv = ep(tc.tile_pool(name="smv", bufs=2))
            abp = ep(tc.tile_pool(name="abp", bufs=2))
            pmm = ep(tc.tile_pool(name="pmm", bufs=5, space="PSUM"))
            ppv = ep(tc.tile_pool(name="ppv", bufs=2, space="PSUM"))
            pst = ep(tc.tile_pool(name="pst", bufs=1, space="PSUM"))
            # ---------- global constants ----------
            ones = cst.tile([128, 1], BF)
            nc.any.memset(ones[:], 1.0)
            epst = cst.tile([1, 1], F32)
            nc.any.memset(epst[:], 1e-5)
            bigm = cst.tile([128, 896], BF, tag="bigm")
            nc.sync.dma_start(bigm[:], masks[:])
            maskt = [bigm[:, 384 - rt * 128:896 - rt * 128] for rt in range(4)]
            fct = cst.tile([128, KT * A_DIM], BF, tag="fcw")
            nc.sync.dma_start(fct[:], fcw[:])
            fcbt = cst.tile([A_DIM, 1], F32, tag="fcb")
            nc.sync.dma_start(fcbt[:], fcb[:].rearrange("(m o) -> m o", o=1))
            wabt = cst.tile([65, E], BF, tag="wab")
            nc.sync.dma_start(wabt[:], wab[:])
            wst = cst.tile([S_DIM, E], BF, tag="ws")
            nc.sync.dma_start(wst[:], wsd[:])
            emb0t = cst.tile([128, 24], F32, tag="emb0")
            nc.sync.dma_start(emb0t[:], emb0[:])

            # residual-stream tiles, two roles that alternate per LN
            xt = [[[xp.tile([128, L], BF, tag=f"x{b}_{j}_{k}", name=f"x{b}_{j}_{k}")
                    for k in range(KT)]
                   for j in range(2)] for b in range(CPC)]
            # attention working set (single set shared by both batches)
            ht = [hp.tile([128, CH], BF, tag=f"h{m}", name=f"h{m}")
                  for m in range(FFT)]

            def ln_chunk(IN, OUT, gt, gnt, bt_, c, post_pos=False):
                """LayerNorm over features for one 512-token chunk."""
                cs = slice(c * CH, (c + 1) * CH)
                st = pst.tile([33, CH], F32, tag="st")
                for k in range(KT):
                    nc.tensor.matmul(st[0:1, :], ones[:], IN[k][:, cs],
                                     start=(k == 0), stop=(k == KT - 1))
                for k in range(KT):
                    xsq = scr.tile([128, CH], BF, tag="xsq", bufs=3)
                    nc.scalar.activation(xsq[:], IN[k][:, cs], AF.Square)
                    nc.tensor.matmul(st[32:33, :], ones[:], xsq[:],
                                     start=(k == 0), stop=(k == KT - 1))
                mu = smv.tile([1, CH], F32, tag="mu", bufs=1)
                nc.vector.tensor_scalar_mul(mu[:], st[0:1, :], 1.0 / D)
                m2 = smv.tile([1, CH], F32, tag="sm", bufs=3)
                nc.vector.tensor_scalar_mul(m2[:], st[32:33, :], 1.0 / D)
                mu2 = smv.tile([1, CH], F32, tag="sm", bufs=3)
                nc.vector.tensor_mul(mu2[:], mu[:], mu[:])
                var = smv.tile([1, CH], F32, tag="sm", bufs=3)
                nc.vector.tensor_sub(var[:], m2[:], mu2[:])
                sd = smv.tile([1, CH], F32, tag="sm", bufs=3)
                nc.scalar.activation(sd[:], var[:], AF.Sqrt, bias=epst[:])
                ab = abp.tile([1, 2 * CH], BF, tag="ab", bufs=1)
                nc.vector.reciprocal(ab[:, 0:CH], sd[:])
                nc.vector.tensor_mul(ab[:, CH:2 * CH], mu[:], ab[:, 0:CH])
                abb = abp.tile([128, 2 * CH], BF, tag="abb", bufs=1)
                nc.gpsimd.partition_broadcast(abb[:], ab[:])
                for k in range(KT):
                    u = scr.tile([128, CH], F32, tag="scr")
                    nc.vector.scalar_tensor_tensor(
                        u[:], IN[k][:, cs], gt[:, k:k + 1], abb[:, 0:CH],
                        op0=AL.mult, op1=AL.mult)
                    w_ = scr.tile([128, CH], F32, tag="scr")
                    nc.vector.scalar_tensor_tensor(
                        w_[:], abb[:, CH:2 * CH], gnt[:, k:k + 1], u[:],
                        op0=AL.mult, op1=AL.add)
                    if post_pos:
                        pe = scr.tile([128, CH], F32, tag="scr")
                        nc.sync.dma_start(pe[:], pos[k * 128:(k + 1) * 128, cs])
                        nc.vector.tensor_add(OUT[k][:, cs], w_[:], pe[:])
                    else:
                        nc.scalar.activation(OUT[k][:, cs], w_[:], AF.Identity,
                                             bias=bt_[:, k:k + 1])

            def emit_forward():
                # ---------- embed + LN + posenc ----------
                def embed_chunk(b, c):
                    cs = slice(c * CH, (c + 1) * CH)
                    ta = scr.tile([65, CH], BF, tag="xsq", bufs=3)
                    nc.sync.dma_start(ta[:], ars[b, 0:65, cs])
                    ts = scr.tile([S_DIM, CH], BF, tag="xsq", bufs=3)
                    nc.sync.dma_start(ts[:], ars[b, 65:193, cs])
                    for m in range(MT):
                        p = pmm.tile([128, CH], F32, tag="mm")
                        ms = slice((m % 2) * 128, (m % 2) * 128 + 128)
                        if m < 2:
                            nc.tensor.matmul(p[:], wabt[0:64, ms], ta[0:64, :],
                                             start=True, stop=True)
                        elif m < 4:
                            nc.tensor.matmul(p[:], wabt[64:65, ms], ta[64:65, :],
                                             start=True, stop=True)
                        else:
                            nc.tensor.matmul(p[:], wst[:, ms], ts[:],
                                             start=True, stop=True)
                        nc.scalar.activation(xt[b][0][m][:, cs], p[:],
                                             AF.Identity,
                                             bias=emb0t[:, m:m + 1])
                    ln_chunk(xt[b][0], xt[b][1], emb0t[:, 6:12],
                             emb0t[:, 12:18], emb0t[:, 18:24], c,
                             post_pos=True)

                # roles: after embed, x lives in role 1
                cur = [1, 1]

                # ---- per-block weight loads; handles kept per block ----
                def wload(i):
                    ws_ = {}
                    for nm, src in (("wq", wq), ("wk", wk), ("wv", wv)):
                        t = wqp.tile([128, KT * D], BF, tag=nm, name=f"{nm}{i}")
                        nc.sync.dma_start(t[:], src[i])
                        ws_[nm] = t
                    return ws_

                def wload2(i, ws_):
                    t = wqp.tile([DH, H * D], BF, tag="wo", name=f"wo{i}")
                    nc.sync.dma_start(t[:], wo[i])
                    ws_["wo"] = t
                    bt = bsp.tile([128, 112], F32, tag="bblk", name=f"bblk{i}")
                    nc.sync.dma_start(bt[:], bblk[i])
                    ws_["bb"] = bt
                    bv = bsp.tile([128, 8 * 97], BF, tag="bvb", name=f"bvb{i}", bufs=1)
                    nc.sync.dma_start(bv[:], bvb[i])
                    ws_["bvb"] = bv

                # attention working-set tiles (allocated fresh per (batch,block))
                def alloc_qkv():
                    qt = [qkp.tile([DH, L], BF, tag=f"q{h}", name=f"q{h}")
                          for h in range(H)]
                    kt_ = [qkp.tile([DH, L], BF, tag=f"k{h}", name=f"k{h}")
                           for h in range(H)]
                    vt = [vap.tile([128, 8 * 97], BF, tag=f"v{tt}", name=f"v{tt}")
                          for tt in range(L // 128)]
                    return qt, kt_, vt

                def qkv_chunk(b, c, ws_, qkvt):
                    """Q/K/V projections for one 512-token chunk."""
                    X = xt[b][cur[b]]
                    qt, kt_, vt = qkvt
                    wqt, wkt, wvt = ws_["wq"], ws_["wk"], ws_["wv"]
                    bb = ws_["bb"]
                    bvbt = ws_["bvb"]
                    cs = slice(c * CH, (c + 1) * CH)
                    for h in range(H):
                        pq = pmm.tile([DH, CH], F32, tag="mm")
                        for k in range(KT):
                            nc.tensor.matmul(
                                pq[:], wqt[:, k * D + h * DH:k * D + (h + 1) * DH],
                                X[k][:, cs], start=(k == 0), stop=(k == KT - 1))
                        nc.vector.tensor_scalar_add(qt[h][:, cs], pq[:],
                                                    bb[0:DH, _BQ + h:_BQ + h + 1])
                        pk = pmm.tile([DH, CH], F32, tag="mm")
                        for k in range(KT):
                            nc.tensor.matmul(
                                pk[:], wkt[:, k * D + h * DH:k * D + (h + 1) * DH],
                                X[k][:, cs], start=(k == 0), stop=(k == KT - 1))
                        nc.vector.tensor_scalar_add(kt_[h][:, cs], pk[:],
                                                    bb[0:DH, _BK + h:_BK + h + 1])
                    for tt in range(CH // 128):
                        tg = c * (CH // 128) + tt
                        tok = slice(tg * 128, (tg + 1) * 128)
                        for hg in range(2):
                            pv = pmm.tile([128, 4 * DH], F32, tag="mm")
                            for k in range(KT):
                                nc.tensor.matmul(
                                    pv[:], X[k][:, tok],
                                    wvt[:, k * D + hg * 4 * DH:k * D + (hg + 1) * 4 * DH],
                                    start=(k == 0), stop=(k == KT - 1))
                            for hh in range(4):
                                h = hg * 4 + hh
                                nc.vector.scalar_tensor_tensor(
                                    vt[tg][:, h * 97:h * 97 + DH],
                                    pv[:, hh * DH:(hh + 1) * DH], 1.0,
                                    bvbt[:, h * 97:h * 97 + DH],
                                    op0=AL.mult, op1=AL.add)
                        nc.vector.tensor_copy(vt[tg][:, 96:8 * 97:97],
                                              bvbt[:, 96:8 * 97:97])

                def att_chunk(b, c, qkvt, ot, zips):
                    """Scores+softmax+PV for one chunk; `zips` is a list of
                    closures (other batch's FFN micro-groups) interleaved after
                    each head's scores so the PE never waits on exp."""
                    qt, kt_, vt = qkvt
                    cs = slice(c * CH, (c + 1) * CH)
                    ktc = 4 * (c + 1)
                    zi = iter(zips)
                    for h in range(H):
                        pts = []
                        for kt2 in range(ktc):
                            ks2 = slice(kt2 * 128, (kt2 + 1) * 128)
                            psc = pmm.tile([128, CH], F32, tag="mm")
                            nc.tensor.matmul(psc[:], kt_[h][:, ks2],
                                             qt[h][:, cs], start=True, stop=True)
                            ptile = ptp.tile([128, CH], BF, tag="pt")
                            nc.scalar.activation(ptile[:], psc[:], AF.Exp)
                            rt = kt2 - 4 * c
                            if rt >= 0:
                                nc.vector.tensor_mul(ptile[:], ptile[:],
                                                     maskt[rt])
                            pts.append(ptile)
                        for _ in range(6):
                            z = next(zi, None)
                            if z is not None:
                                z()
                        po = ppv.tile([DH + 1, CH], F32, tag="pv")
                        for kt2 in range(ktc):
                            nc.tensor.matmul(
                                po[:], vt[kt2][:, h * 97:h * 97 + 97], pts[kt2][:],
                                start=(kt2 == 0), stop=(kt2 == ktc - 1))
                        dinv = abp.tile([1, CH], BF, tag="dinv", name="dinv", bufs=1)
                        nc.vector.reciprocal(dinv[:], po[DH:DH + 1, :])
                        dib = abp.tile([DH, CH], BF, tag="dib", name="dib", bufs=1)
                        nc.gpsimd.partition_broadcast(dib[:], dinv[:])
                        oht = otp.tile([DH, CH], BF, tag=f"o{h}", name=f"o{h}")
                        nc.vector.scalar_tensor_tensor(
                            oht[:], po[0:DH, :], 1.0, dib[:],
                            op0=AL.mult, op1=AL.mult)
                        ot[h] = oht
                    for z in zi:
                        z()

                def o_chunk(b, c, ws_, ot):
                    X = xt[b][cur[b]]
                    R = xt[b][1 - cur[b]]
                    wot = ws_["wo"]
                    bb = ws_["bb"]
                    cs = slice(c * CH, (c + 1) * CH)
                    for m in range(MT):
                        ms = slice(m * 128, (m + 1) * 128)
                        pp = pmm.tile([128, CH], F32, tag="mm")
                        for h in range(H):
                            nc.tensor.matmul(pp[:],
                                             wot[:, h * D + m * 128:h * D + (m + 1) * 128],
                                             ot[h][:], start=(h == 0),
                                             stop=(h == H - 1))
                        nc.vector.scalar_tensor_tensor(
                            R[m][:, cs], pp[:], bb[:, _BO + m:_BO + m + 1],
                            X[m][:, cs], op0=AL.add, op1=AL.add)

                def ffn_micros(b, c, i):
                    """FFN for one chunk as 16 closures (8 ffn1 + 8 ffn2)."""
                    X = xt[b][cur[b]]
                    R = xt[b][1 - cur[b]]

                    def f1(m, ws_):
                        def run():
                            wt = wsp.tile([128, KT * 128], BF, tag="wst",
                                          name=f"w1_{m}")
                            nc.sync.dma_start(wt[:], w1[i, :, m, :])
                            p1 = pmm.tile([128, CH], F32, tag="mm")
                            for k in range(KT):
                                nc.tensor.matmul(
                                    p1[:], wt[:, k * 128:(k + 1) * 128],
                                    R[k][:, c * CH:(c + 1) * CH],
                                    start=(k == 0), stop=(k == KT - 1))
                            nc.scalar.activation(
                                ht[m][:], p1[:], AF.Relu,
                                bias=ws_["bb"][:, _B1 + m:_B1 + m + 1])
                        return run

                    p2s = {}

                    def f2(g, kq, ws_):
                        def run():
                            wt = wsp.tile([128, 2 * 384], BF, tag="wst",
                                          name=f"w2_{g}_{kq}")
                            nc.sync.dma_start(
                                wt[:], w2[i, :, kq * 2:(kq + 1) * 2,
                                           g * 384:(g + 1) * 384])
                            if kq == 0:
                                p2s[g] = [pmm.tile([128, CH], F32, tag="mm",
                                                   name=f"p2_{g}_{mi}")
                                          for mi in range(3)]
                            for kk in range(2):
                                k = kq * 2 + kk
                                for mi in range(3):
                                    nc.tensor.matmul(
                                        p2s[g][mi][:],
                                        wt[:, kk * 384 + mi * 128:kk * 384 + (mi + 1) * 128],
                                        ht[k][:], start=(k == 0),
                                        stop=(k == FFT - 1))
                            if kq == 11:
                                cs = slice(c * CH, (c + 1) * CH)
                                for mi in range(3):
                                    m = g * 3 + mi
                                    nc.vector.scalar_tensor_tensor(
                                        X[m][:, cs], p2s[g][mi][:],
                                        ws_["bb"][:, _B2 + m:_B2 + m + 1],
                                        R[m][:, cs], op0=AL.add, op1=AL.add)
                        return run
                    return f1, f2

                def ffn_micro_list(b, c, i, ws_):
                    f1, f2 = ffn_micros(b, c, i)
                    return ([f1(m, ws_) for m in range(FFT)] +
                            [f2(g, kq, ws_) for g in range(2) for kq in range(12)])

                def ln_stage(b, which, ws_, c):
                    bb = ws_["bb"]
                    X = xt[b][cur[b]]
                    R = xt[b][1 - cur[b]]
                    if which == 1:
                        ln_chunk(R, X, bb[:, _L1G:_L1G + 6], bb[:, _L1N:_L1N + 6],
                                 bb[:, _CAB + b * 6:_CAB + (b + 1) * 6], c)
                    elif which == 2:
                        ln_chunk(X, R, bb[:, _L2G:_L2G + 6], bb[:, _L2N:_L2N + 6],
                                 bb[:, _L2B:_L2B + 6], c)
                    else:
                        ln_chunk(X, R, bb[:, _L3G:_L3G + 6], bb[:, _L3N:_L3N + 6],
                                 bb[:, _L3B:_L3B + 6], c)

                def head_chunk(b, c):
                    Xf = xt[b][cur[b]]
                    cs = slice(c * CH, (c + 1) * CH)
                    pf = pmm.tile([A_DIM, CH], F32, tag="mm")
                    for k in range(KT):
                        nc.tensor.matmul(pf[:], fct[:, k * A_DIM:(k + 1) * A_DIM],
                                         Xf[k][:, cs], start=(k == 0),
                                         stop=(k == KT - 1))
                    yt = scr.tile([A_DIM, CH], F32, tag="scr")
                    nc.vector.tensor_scalar_add(yt[:], pf[:], fcbt[:])
                    nc.sync.dma_start(y[b, :, cs], yt[:])

                # ---------- transformer blocks: 2-batch pipeline ----------
                WS = wload(0)
                wload2(0, WS)
                WSn = None
                qkv_b = [None, None]   # live qkv tiles per batch
                # prologue: embeds, then b0's QKV (overlaps embed LN tails)
                embed_chunk(0, 0)
                embed_chunk(0, 1)
                embed_chunk(1, 0)
                embed_chunk(1, 1)
                qkv_b[0] = alloc_qkv()
                qkv_chunk(0, 0, WS, qkv_b[0])
                qkv_chunk(0, 1, WS, qkv_b[0])
                ot_b = [[None] * H, [None] * H]
                pend = None            # (b, ws, micros c0, micros c1) FFN of
                                       # other batch from previous block
                for i in range(NB):
                    for b in (0, 1):
                        o = 1 - b
                        if b == 0 and i > 0:
                            WS = WSn
                        if qkv_b[b] is None:  # prologue only (b0, block 0)
                            qkv_b[b] = alloc_qkv()
                            for c in range(NCH):
                                qkv_chunk(b, c, WS, qkv_b[b])
                        # attention zipped with other batch's pending FFN
                        if pend is not None:
                            pb, pws, pm0, pm1 = pend
                            att_chunk(b, 0, qkv_b[b], ot_b[b], pm0)
                            o_chunk(b, 0, WS, ot_b[b])
                            att_chunk(b, 1, qkv_b[b], ot_b[b], pm1)
                            ln_stage(pb, 3, pws, 0)
                            o_chunk(b, 1, WS, ot_b[b])
                            ln_stage(pb, 3, pws, 1)
                            cur[pb] = 1 - cur[pb]
                        else:
                            att_chunk(b, 0, qkv_b[b], ot_b[b], [])
                            o_chunk(b, 0, WS, ot_b[b])
                            att_chunk(b, 1, qkv_b[b], ot_b[b], [])
                            o_chunk(b, 1, WS, ot_b[b])
                        # LN1 zipped with other batch's QKV (next user of the
                        # single qkv tile set)
                        ln_stage(b, 1, WS, 0)
                        if b == 0:
                            # b1's qkv for this block
                            qkv_b[o] = alloc_qkv()
                            qkv_chunk(o, 0, WS, qkv_b[o])
                            ln_stage(b, 1, WS, 1)
                            qkv_chunk(o, 1, WS, qkv_b[o])
                            WSn = wload(i + 1) if i + 1 < NB else None
                        else:
                            if i + 1 < NB:
                                # b0's qkv for next block (uses next weights)
                                wload2(i + 1, WSn)
                                qkv_b[o] = alloc_qkv()
                                qkv_chunk(o, 0, WSn, qkv_b[o])
                                ln_stage(b, 1, WS, 1)
                                qkv_chunk(o, 1, WSn, qkv_b[o])
                            else:
                                ln_stage(b, 1, WS, 1)
                                qkv_b[o] = None
                        ln_stage(b, 2, WS, 0)
                        ln_stage(b, 2, WS, 1)
                        pend = (b, WS, ffn_micro_list(b, 0, i, WS),
                                ffn_micro_list(b, 1, i, WS))

                # ---------- epilogue: b1's last FFN + LN3, then head ----------
                pb, pws, pm0, pm1 = pend
                for z in pm0:
                    z()
                head_chunk(0, 0)
                for z in pm1:
                    z()
                head_chunk(0, 1)
                ln_stage(pb, 3, pws, 0)
                ln_stage(pb, 3, pws, 1)
                cur[1] = 1 - cur[1]
                head_chunk(1, 0)
                head_chunk(1, 1)

            for _rep in range(reps):
                emit_forward()

    nc.compile()
    return nc


def _posenc(length, d):
    pos_ = np.arange(length, dtype=np.float32)[:, None]
    i = np.arange(0, d, 2, dtype=np.float32)[None, :]
    ang = pos_ / np.power(np.float32(10000.0), i / np.float32(d))
    pe = np.zeros((length, d), np.float32)
    pe[:, 0::2] = np.sin(ang)
    pe[:, 1::2] = np.cos(ang)
    return pe


def _pack_pk(w, p=128):
    """[K*p, M] -> [p, K*M] partition-major packing."""
    k = w.shape[0] // p
    return np.ascontiguousarray(
        w.reshape(k, p, w.shape[1]).transpose(1, 0, 2).reshape(p, -1))


def _host_prep(inp):
    f32 = np.float32
    a, r, s, t = (np.asarray(inp[k]) for k in ("a", "r", "s", "t"))
    ars = np.concatenate(
        [np.asarray(a, f32), np.asarray(r, f32), np.asarray(s, f32)],
        axis=-1).transpose(0, 2, 1)  # [B, 193, L]
    ars = np.ascontiguousarray(ars).astype(bf)

    scale = f32(1.0 / np.sqrt(DH))
    sa_Wqkv = np.asarray(inp["sa_Wqkv"], f32)
    sa_bqkv = np.asarray(inp["sa_bqkv"], f32)
    wq_p = np.stack([_pack_pk((sa_Wqkv[i, 0] * scale).astype(bf))
                     for i in range(NB)])
    wk_p = np.stack([_pack_pk(sa_Wqkv[i, 1].astype(bf)) for i in range(NB)])
    wv_p = np.stack([_pack_pk(sa_Wqkv[i, 2].astype(bf)) for i in range(NB)])
    wo_p = np.stack([_pack_pk(np.asarray(inp["sa_Wo"], f32)[i].astype(bf), p=DH)
                     for i in range(NB)])
    w1_p = np.stack([
        np.ascontiguousarray(
            np.asarray(inp["ff_W1"], f32)[i].astype(bf)
            .reshape(KT, 128, FFT, 128).transpose(1, 2, 0, 3)
            .reshape(128, FFT, KT * 128))
        for i in range(NB)])
    w2_p = np.stack([
        _pack_pk(np.asarray(inp["ff_W2"], f32)[i].astype(bf)).reshape(
            128, FFT, D) for i in range(NB)])

    bq = sa_bqkv[:, 0] * scale
    bk = sa_bqkv[:, 1]
    bv = sa_bqkv[:, 2]
    bvb = np.zeros((NB, 128, 8 * 97), f32)
    for h in range(H):
        bvb[:, :, h * 97:h * 97 + DH] = bv[:, None, h * DH:(h + 1) * DH]
        bvb[:, :, h * 97 + DH] = 1.0
    pcol = np.arange(128)[:, None]
    ucol = np.arange(896)[None, :]
    masks = np.where(pcol > ucol - 384, f32(0.0), f32(1.0))

    task_table = np.asarray(inp["task_table"], f32)
    ca_Wqkv = np.asarray(inp["ca_Wqkv"], f32)
    ca_bqkv = np.asarray(inp["ca_bqkv"], f32)
    ca_Wo = np.asarray(inp["ca_Wo"], f32)
    ca_bo = np.asarray(inp["ca_bo"], f32)
    ln1_b = np.asarray(inp["ln1_b"], f32)
    enc = task_table[np.asarray(t)[:, 0]]  # [B, D]
    cab = np.zeros((NB, B, D), f32)
    for i in range(NB):
        v_ = enc @ ca_Wqkv[i, 2] + ca_bqkv[i, 2]
        cab[i] = v_ @ ca_Wo[i] + ca_bo[i]
    cabb_all = cab + ln1_b[:, None, :]  # [NB, B, D]

    def cols(x, p=128):
        # [n*p] -> [p, n] column pack
        return x.reshape(-1, p).T

    ln1_g = np.asarray(inp["ln1_g"], f32)
    l2g = np.asarray(inp["ln2_g"], f32)
    l2b = np.asarray(inp["ln2_b"], f32)
    l3g = np.asarray(inp["ln3_g"], f32)
    l3b = np.asarray(inp["ln3_b"], f32)
    bo_ = np.asarray(inp["sa_bo"], f32)
    b1_ = np.asarray(inp["ff_b1"], f32)
    b2_ = np.asarray(inp["ff_b2"], f32)

    bblk_shared = np.zeros((NB, 128, 112), f32)
    for i in range(NB):
        bblk_shared[i, 0:DH, _BQ:_BQ + 8] = bq[i].reshape(H, DH).T
        bblk_shared[i, 0:DH, _BK:_BK + 8] = bk[i].reshape(H, DH).T
        bblk_shared[i, :, _BO:_BO + 6] = cols(bo_[i])
        bblk_shared[i, :, _B1:_B1 + 24] = cols(b1_[i])
        bblk_shared[i, :, _B2:_B2 + 6] = cols(b2_[i])
        bblk_shared[i, :, _L1G:_L1G + 6] = cols(ln1_g[i])
        bblk_shared[i, :, _L1N:_L1N + 6] = cols(-ln1_g[i])
        bblk_shared[i, :, _L2G:_L2G + 6] = cols(l2g[i])
        bblk_shared[i, :, _L2N:_L2N + 6] = cols(-l2g[i])
        bblk_shared[i, :, _L2B:_L2B + 6] = cols(l2b[i])
        bblk_shared[i, :, _L3G:_L3G + 6] = cols(l3g[i])
        bblk_shared[i, :, _L3N:_L3N + 6] = cols(-l3g[i])
        bblk_shared[i, :, _L3B:_L3B + 6] = cols(l3b[i])

    ln_g = np.asarray(inp["ln_g"], f32)
    ln_b = np.asarray(inp["ln_b"], f32)
    bemb = np.concatenate([np.asarray(inp["ba"], f32),
                           np.asarray(inp["br"], f32),
                           np.asarray(inp["bs"], f32)])
    emb0 = np.zeros((128, 24), f32)
    emb0[:, 0:6] = cols(bemb)
    emb0[:, 6:12] = cols(ln_g)
    emb0[:, 12:18] = cols(-ln_g)
    emb0[:, 18:24] = cols(ln_b)

    wab = np.concatenate([np.asarray(inp["Wa"], f32),
                          np.asarray(inp["Wr"], f32)], axis=0)  # [65, E]
    fcw_p = _pack_pk(np.asarray(inp["fc_W"], f32).astype(bf))  # [128, 6*64]

    shared = dict(
        wab=wab.astype(bf),
        wsd=np.asarray(inp["Ws"], f32).astype(bf),
        emb0=emb0,
        pos=np.ascontiguousarray(_posenc(L, D).T
                                 + ln_b[:, None]),
        wq=wq_p, wk=wk_p, wv=wv_p, wo=wo_p, w1=w1_p, w2=w2_p,
        bvb=bvb.astype(bf),
        masks=masks.astype(bf),
        fcw=fcw_p,
        fcb=np.asarray(inp["fc_b"], f32),
    )
    in_maps = []
    for core in range(NCORES):
        m = dict(shared)
        m["ars"] = ars[core * CPC:(core + 1) * CPC]
        bb = bblk_shared.copy()
        for i in range(NB):
            for b in range(CPC):
                bb[i, :, _CAB + b * 6:_CAB + (b + 1) * 6] = cols(
                    cabb_all[i, core * CPC + b])
        m["bblk"] = bb
        in_maps.append(m)
    return in_maps


def _get_nc(reps=1):
    key = f"nc{reps}"
    if key not in _CACHE:
        _CACHE[key] = _build(reps)
    return _CACHE[key]


def kernel(**inputs):
    nc = _get_nc()
    in_maps = _host_prep(inputs)
    res = run_bass_kernel_spmd(nc, in_maps, core_ids=list(range(NCORES)))
    out = np.zeros((B, L, A_DIM), np.float32)
    for core in range(NCORES):
        yc = res.results[core]["y"]  # [CPC, 64, L]
        for b in range(CPC):
            out[core * CPC + b] = yc[b].T
    return out


# revision 33
# speedup vs baseline: 16.1267x; 9.6794x over previous
"""Trainium2 Bass kernel for nn_DiscreteDecisionTransformer.

Decision-transformer forward: embed(a,r,s) -> LN -> +posenc, then 4 blocks of
[causal self-attn, cross-attn, FFN] with post-LN, then action head.

Distribution: data-parallel over batch, 16 batches / 8 cores = 2 per core.
Params replicated; zero collectives. Feature-major activations ([dmodel on
partitions, tokens on free dim]) so GEMMs contract over partitions.

v2 changes vs baseline:
 - Two-batch software pipeline: batch A's attention (exp/softmax on Act/DVE)
   is interleaved at head/micro-group granularity with batch B's FFN (PE
   heavy), so the PE never waits on softmax or LayerNorm chains.
 - All weights host-packed partition-major and DMA'd in large merged
   transfers (~200 DMAs/core vs ~2200): HWDGE descriptor-generation was a
   serialized 1.4ms in the baseline.
 - Per-block scalar params packed into one [128,112] f32 tensor (1 DMA).

Host prep (unchanged math): cross-attn collapses to a per-(block,batch) bias
fused into LN1 beta; 1/sqrt(dh) folded into Wq; additive causal mask tiles;
softmax denominators via ones-column appended to V (97-col heads); LN stats
(sum, sumsq) on the PE with a ones-vector lhsT.

GEMMs in bf16 with f32 PSUM accumulation.
"""

import sys
from contextlib import ExitStack

sys.path.insert(0, "/opt/trn_rl_repo")

import numpy as np
import ml_dtypes

import concourse.bacc as bacc
import concourse.mybir as mybir
import concourse.tile as tile
from concourse.bass_utils import run_bass_kernel_spmd

bf = ml_dtypes.bfloat16

B, L, D, H, DH, NB, E = 16, 1024, 768, 8, 96, 4, 256
A_DIM, S_DIM = 64, 128
NCORES = 8
CPC = B // NCORES  # batches per core
KT = D // 128      # 6 k-tiles of dmodel
MT = D // 128      # 6 m-tiles of dmodel
CH = 512           # token chunk (matmul N)
NCH = L // CH      # 2 chunks per batch
FFT = 4 * D // 128 # 24 m-tiles of ffn hidden
F32, BF = mybir.dt.float32, mybir.dt.bfloat16
AL = mybir.AluOpType
AF = mybir.ActivationFunctionType

# biasblk column layout
_BQ, _BK, _BO, _B1, _B2, _CAB, _L1G, _L1N = 0, 8, 16, 22, 46, 52, 64, 70
_L2G, _L2N, _L2B, _L3G, _L3N, _L3B = 76, 82, 88, 94, 100, 106

_CACHE = {}


def _build(reps=1):
    """Emit the full per-core program. Returns the finished Bacc object."""
    nc = bacc.Bacc("TRN2", target_bir_lowering=False, debug=False)
    dram = nc.dram_tensor

    ars = dram("ars", [CPC, 193, L], BF, kind="ExternalInput")
    wab = dram("wab", [65, E], BF, kind="ExternalInput")
    wsd = dram("wsd", [S_DIM, E], BF, kind="ExternalInput")
    emb0 = dram("emb0", [128, 24], F32, kind="ExternalInput")
    pos = dram("pos", [D, L], F32, kind="ExternalInput")
    wq = dram("wq", [NB, 128, KT * D], BF, kind="ExternalInput")
    wk = dram("wk", [NB, 128, KT * D], BF, kind="ExternalInput")
    wv = dram("wv", [NB, 128, KT * D], BF, kind="ExternalInput")
    wo = dram("wo", [NB, DH, H * D], BF, kind="ExternalInput")
    w1 = dram("w1", [NB, 128, FFT, KT * 128], BF, kind="ExternalInput")
    w2 = dram("w2", [NB, 128, FFT, D], BF, kind="ExternalInput")
    bvb = dram("bvb", [NB, 128, 8 * 97], BF, kind="ExternalInput")
    bblk = dram("bblk", [NB, 128, 112], F32, kind="ExternalInput")
    masks = dram("masks", [128, 896], BF, kind="ExternalInput")
    fcw = dram("fcw", [128, KT * A_DIM], BF, kind="ExternalInput")
    fcb = dram("fcb", [A_DIM], F32, kind="ExternalInput")
    y = dram("y", [CPC, A_DIM, L], F32, kind="ExternalOutput")

    with nc.allow_low_precision(reason="bf16 kernel by design"), \
         tile.TileContext(nc) as tc, ExitStack() as ctx:
            ep = ctx.enter_context
            cst = ep(tc.tile_pool(name="cst", bufs=1))
            wqp = ep(tc.tile_pool(name="wqp", bufs=1))
            wsp = ep(tc.tile_pool(name="wsp", bufs=6))
            bsp = ep(tc.tile_pool(name="bsp", bufs=2))
            xp = ep(tc.tile_pool(name="xp", bufs=1))
            qkp = ep(tc.tile_pool(name="qk", bufs=1))
            vap = ep(tc.tile_pool(name="vap", bufs=1))
            ptp = ep(tc.tile_pool(name="ptp", bufs=8))
            otp = ep(tc.tile_pool(name="otp", bufs=1))
            scr = ep(tc.tile_pool(name="scr", bufs=2))
            hp = ep(tc.tile_pool(name="hp", bufs=1))
            smv = ep(tc.tile_pool(name="smv", bufs=2))
            abp = ep(tc.tile_pool(name="abp", bufs=2))
            pmm = ep(tc.tile_pool(name="pmm", bufs=6, space="PSUM"))
            ppv = ep(tc.tile_pool(name="ppv", bufs=1, space="PSUM"))
            pst = ep(tc.tile_pool(name="pst", bufs=1, space="PSUM"))
            # ---------- global constants ----------
            ones = cst.tile([128, 1], BF)
            nc.any.memset(ones[:], 1.0)
            epst = cst.tile([1, 1], F32)
            nc.any.memset(epst[:], 1e-5)
            bigm = cst.tile([128, 896], BF, tag="bigm")
            maskt = [bigm[:, 384 - rt * 128:896 - rt * 128] for rt in range(4)]
            fct = cst.tile([128, KT * A_DIM], BF, tag="fcw")
            fcbt = cst.tile([A_DIM, 1], F32, tag="fcb")
            wabt = cst.tile([65, E], BF, tag="wab")
            nc.sync.dma_start(wabt[:], wab[:])
            wst = cst.tile([S_DIM, E], BF, tag="ws")
            nc.sync.dma_start(wst[:], wsd[:])
            emb0t = cst.tile([128, 24], F32, tag="emb0")
            nc.sync.dma_start(emb0t[:], emb0[:])

            # residual-stream tiles, two roles that alternate per LN
            xt = [[[xp.tile([128, L], BF, tag=f"x{b}_{j}_{k}", name=f"x{b}_{j}_{k}")
                    for k in range(KT)]
                   for j in range(2)] for b in range(CPC)]
            # attention working set (single set shared by both batches)
            ht = [hp.tile([128, CH], BF, tag=f"h{m}", name=f"h{m}")
                  for m in range(FFT)]

            def ln_chunk(IN, OUT, gt, gnt, bt_, c, post_pos=False):
                """LayerNorm over features for one 512-token chunk."""
                cs = slice(c * CH, (c + 1) * CH)
                st = pst.tile([33, CH], F32, tag="st")
                for k in range(KT):
                    nc.tensor.matmul(st[0:1, :], ones[:], IN[k][:, cs],
                                     start=(k == 0), stop=(k == KT - 1))
                for k in range(KT):
                    xsq = scr.tile([128, CH], BF, tag="xsq", bufs=3)
                    nc.scalar.activation(xsq[:], IN[k][:, cs], AF.Square)
                    nc.tensor.matmul(st[32:33, :], ones[:], xsq[:],
                                     start=(k == 0), stop=(k == KT - 1))
                mu = smv.tile([1, CH], F32, tag="mu", bufs=1)
                nc.vector.tensor_scalar_mul(mu[:], st[0:1, :], 1.0 / D)
                m2 = smv.tile([1, CH], F32, tag="sm", bufs=3)
                nc.vector.tensor_scalar_mul(m2[:], st[32:33, :], 1.0 / D)
                mu2 = smv.tile([1, CH], F32, tag="sm", bufs=3)
                nc.vector.tensor_mul(mu2[:], mu[:], mu[:])
                var = smv.tile([1, CH], F32, tag="sm", bufs=3)
                nc.vector.tensor_sub(var[:], m2[:], mu2[:])
                sd = smv.tile([1, CH], F32, tag="sm", bufs=3)
                nc.scalar.activation(sd[:], var[:], AF.Sqrt, bias=epst[:])
                ab = abp.tile([1, 2 * CH], BF, tag="ab", bufs=1)
                nc.vector.reciprocal(ab[:, 0:CH], sd[:])
                nc.vector.tensor_mul(ab[:, CH:2 * CH], mu[:], ab[:, 0:CH])
                abb = abp.tile([128, 2 * CH], BF, tag="abb", bufs=1)
                nc.gpsimd.partition_broadcast(abb[:], ab[:])
                for k in range(KT):
                    u = scr.tile([128, CH], F32, tag="scr")
                    nc.vector.scalar_tensor_tensor(
                        u[:], IN[k][:, cs], gt[:, k:k + 1], abb[:, 0:CH],
                        op0=AL.mult, op1=AL.mult)
                    w_ = scr.tile([128, CH], F32, tag="scr")
                    nc.vector.scalar_tensor_tensor(
                        w_[:], abb[:, CH:2 * CH], gnt[:, k:k + 1], u[:],
                        op0=AL.mult, op1=AL.add)
                    if post_pos:
                        pe = scr.tile([128, CH], F32, tag="scr")
                        nc.sync.dma_start(pe[:], pos[k * 128:(k + 1) * 128, cs])
                        nc.vector.tensor_add(OUT[k][:, cs], w_[:], pe[:])
                    else:
                        nc.scalar.activation(OUT[k][:, cs], w_[:], AF.Identity,
                                             bias=bt_[:, k:k + 1])

            def emit_forward():
                # ---------- embed + LN + posenc ----------
                def embed_chunk(b, c):
                    cs = slice(c * CH, (c + 1) * CH)
                    ta = scr.tile([65, CH], BF, tag="xsq", bufs=3)
                    nc.sync.dma_start(ta[:], ars[b, 0:65, cs])
                    ts = scr.tile([S_DIM, CH], BF, tag="xsq", bufs=3)
                    nc.sync.dma_start(ts[:], ars[b, 65:193, cs])
                    for m in range(MT):
                        p = pmm.tile([128, CH], F32, tag="mm")
                        ms = slice((m % 2) * 128, (m % 2) * 128 + 128)
                        if m < 2:
                            nc.tensor.matmul(p[:], wabt[0:64, ms], ta[0:64, :],
                                             start=True, stop=True)
                        elif m < 4:
                            nc.tensor.matmul(p[:], wabt[64:65, ms], ta[64:65, :],
                                             start=True, stop=True)
                        else:
                            nc.tensor.matmul(p[:], wst[:, ms], ts[:],
                                             start=True, stop=True)
                        nc.scalar.activation(xt[b][0][m][:, cs], p[:],
                                             AF.Identity,
                                             bias=emb0t[:, m:m + 1])
                    ln_chunk(xt[b][0], xt[b][1], emb0t[:, 6:12],
                             emb0t[:, 12:18], emb0t[:, 18:24], c,
                             post_pos=True)

                # roles: after embed, x lives in role 1
                cur = [1, 1]

                # ---- per-block weight loads; handles kept per block ----
                def wload(i):
                    ws_ = {}
                    for nm, src in (("wq", wq), ("wk", wk), ("wv", wv)):
                        t = wqp.tile([128, KT * D], BF, tag=nm, name=f"{nm}{i}")
                        nc.sync.dma_start(t[:], src[i])
                        ws_[nm] = t
                    return ws_

                def wload2(i, ws_):
                    t = wqp.tile([DH, H * D], BF, tag="wo", name=f"wo{i}")
                    nc.sync.dma_start(t[:], wo[i])
                    ws_["wo"] = t
                    bt = bsp.tile([128, 112], F32, tag="bblk", name=f"bblk{i}")
                    nc.sync.dma_start(bt[:], bblk[i])
                    ws_["bb"] = bt
                    bv = bsp.tile([128, 8 * 97], BF, tag="bvb", name=f"bvb{i}", bufs=1)
                    nc.sync.dma_start(bv[:], bvb[i])
                    ws_["bvb"] = bv

                # attention working-set tiles (allocated fresh per (batch,block))
                def alloc_qkv():
                    qt = [qkp.tile([DH, L], BF, tag=f"q{h}", name=f"q{h}")
                          for h in range(H)]
                    kt_ = [qkp.tile([DH, L], BF, tag=f"k{h}", name=f"k{h}")
                           for h in range(H)]
                    vt = [vap.tile([128, 8 * 97], BF, tag=f"v{tt}", name=f"v{tt}")
                          for tt in range(L // 128)]
                    return qt, kt_, vt

                def qkv_chunk(b, c, ws_, qkvt):
                    """Q/K/V projections for one 512-token chunk."""
                    X = xt[b][cur[b]]
                    qt, kt_, vt = qkvt
                    wqt, wkt, wvt = ws_["wq"], ws_["wk"], ws_["wv"]
                    bb = ws_["bb"]
                    bvbt = ws_["bvb"]
                    cs = slice(c * CH, (c + 1) * CH)
                    for h in range(H):
                        pq = pmm.tile([DH, CH], F32, tag="mm")
                        for k in range(KT):
                            nc.tensor.matmul(
                                pq[:], wqt[:, k * D + h * DH:k * D + (h + 1) * DH],
                                X[k][:, cs], start=(k == 0), stop=(k == KT - 1))
                        nc.vector.tensor_scalar_add(qt[h][:, cs], pq[:],
                                                    bb[0:DH, _BQ + h:_BQ + h + 1])
                        pk = pmm.tile([DH, CH], F32, tag="mm")
                        for k in range(KT):
                            nc.tensor.matmul(
                                pk[:], wkt[:, k * D + h * DH:k * D + (h + 1) * DH],
                                X[k][:, cs], start=(k == 0), stop=(k == KT - 1))
                        nc.vector.tensor_scalar_add(kt_[h][:, cs], pk[:],
                                                    bb[0:DH, _BK + h:_BK + h + 1])
                    for tt in range(CH // 128):
                        tg = c * (CH // 128) + tt
                        tok = slice(tg * 128, (tg + 1) * 128)
                        for hg in range(2):
                            pv = pmm.tile([128, 4 * DH], F32, tag="mm")
                            for k in range(KT):
                                nc.tensor.matmul(
                                    pv[:], X[k][:, tok],
                                    wvt[:, k * D + hg * 4 * DH:k * D + (hg + 1) * 4 * DH],
                                    start=(k == 0), stop=(k == KT - 1))
                            for hh in range(4):
                                h = hg * 4 + hh
                                nc.vector.scalar_tensor_tensor(
                                    vt[tg][:, h * 97:h * 97 + DH],
                                    pv[:, hh * DH:(hh + 1) * DH], 1.0,
                                    bvbt[:, h * 97:h * 97 + DH],
                                    op0=AL.mult, op1=AL.add)
                        nc.vector.tensor_copy(vt[tg][:, 96:8 * 97:97],
                                              bvbt[:, 96:8 * 97:97])

                def att_chunk(b, c, qkvt, ot, zips):
                    """Scores+softmax+PV for one chunk; `zips` is a list of
                    closures (other batch's FFN micro-groups) interleaved after
                    each head's scores so the PE never waits on exp."""
                    qt, kt_, vt = qkvt
                    cs = slice(c * CH, (c + 1) * CH)
                    ktc = 4 * (c + 1)
                    zi = iter(zips)
                    for h in range(H):
                        pts = []
                        for kt2 in range(ktc):
                            ks2 = slice(kt2 * 128, (kt2 + 1) * 128)
                            psc = pmm.tile([128, CH], F32, tag="mm")
                            nc.tensor.matmul(psc[:], kt_[h][:, ks2],
                                             qt[h][:, cs], start=True, stop=True)
                            ptile = ptp.tile([128, CH], BF, tag="pt")
                            nc.scalar.activation(ptile[:], psc[:], AF.Exp)
                            rt = kt2 - 4 * c
                            if rt >= 0:
                                nc.vector.tensor_mul(ptile[:], ptile[:],
                                                     maskt[rt])
                            pts.append(ptile)
                        for _ in range(6):
                            z = next(zi, None)
                            if z is not None:
                                z()
                        po = ppv.tile([DH + 1, CH], F32, tag="pv")
                        for kt2 in range(ktc):
                            nc.tensor.matmul(
                                po[:], vt[kt2][:, h * 97:h * 97 + 97], pts[kt2][:],
                                start=(kt2 == 0), stop=(kt2 == ktc - 1))
                        dinv = abp.tile([1, CH], BF, tag="dinv", name="dinv", bufs=1)
                        nc.vector.reciprocal(dinv[:], po[DH:DH + 1, :])
                        dib = abp.tile([DH, CH], BF, tag="dib", name="dib", bufs=1)
                        nc.gpsimd.partition_broadcast(dib[:], dinv[:])
                        oht = otp.tile([DH, CH], BF, tag=f"o{h}", name=f"o{h}")
                        nc.vector.scalar_tensor_tensor(
                            oht[:], po[0:DH, :], 1.0, dib[:],
                            op0=AL.mult, op1=AL.mult)
                        ot[h] = oht
                    for z in zi:
                        z()

                def o_chunk(b, c, ws_, ot):
                    X = xt[b][cur[b]]
                    R = xt[b][1 - cur[b]]
                    wot = ws_["wo"]
                    bb = ws_["bb"]
                    cs = slice(c * CH, (c + 1) * CH)
                    for m in range(MT):
                        ms = slice(m * 128, (m + 1) * 128)
                        pp = pmm.tile([128, CH], F32, tag="mm")
                        for h in range(H):
                            nc.tensor.matmul(pp[:],
                                             wot[:, h * D + m * 128:h * D + (m + 1) * 128],
                                             ot[h][:], start=(h == 0),
                                             stop=(h == H - 1))
                        nc.vector.scalar_tensor_tensor(
                            R[m][:, cs], pp[:], bb[:, _BO + m:_BO + m + 1],
                            X[m][:, cs], op0=AL.add, op1=AL.add)

                def ffn_micros(b, c, i):
                    """FFN for one chunk as 16 closures (8 ffn1 + 8 ffn2)."""
                    X = xt[b][cur[b]]
                    R = xt[b][1 - cur[b]]

                    def f1(m, ws_):
                        def run():
                            wt = wsp.tile([128, KT * 128], BF, tag="wst",
                                          name=f"w1_{m}")
                            nc.sync.dma_start(wt[:], w1[i, :, m, :])
                            p1 = pmm.tile([128, CH], F32, tag="mm")
                            for k in range(KT):
                                nc.tensor.matmul(
                                    p1[:], wt[:, k * 128:(k + 1) * 128],
                                    R[k][:, c * CH:(c + 1) * CH],
                                    start=(k == 0), stop=(k == KT - 1))
                            nc.scalar.activation(
                                ht[m][:], p1[:], AF.Relu,
                                bias=ws_["bb"][:, _B1 + m:_B1 + m + 1])
                        return run

                    p2s = {}

                    def f2(g, kq, ws_):
                        def run():
                            wt = wsp.tile([128, 2 * 384], BF, tag="wst",
                                          name=f"w2_{g}_{kq}")
                            nc.sync.dma_start(
                                wt[:], w2[i, :, kq * 2:(kq + 1) * 2,
                                           g * 384:(g + 1) * 384])
                            if kq == 0:
                                p2s[g] = [pmm.tile([128, CH], F32, tag="mm",
                                                   name=f"p2_{g}_{mi}")
                                          for mi in range(3)]
                            for kk in range(2):
                                k = kq * 2 + kk
                                for mi in range(3):
                                    nc.tensor.matmul(
                                        p2s[g][mi][:],
                                        wt[:, kk * 384 + mi * 128:kk * 384 + (mi + 1) * 128],
                                        ht[k][:], start=(k == 0),
                                        stop=(k == FFT - 1))
                            if kq == 11:
                                cs = slice(c * CH, (c + 1) * CH)
                                for mi in range(3):
                                    m = g * 3 + mi
                                    nc.vector.scalar_tensor_tensor(
                                        X[m][:, cs], p2s[g][mi][:],
                                        ws_["bb"][:, _B2 + m:_B2 + m + 1],
                                        R[m][:, cs], op0=AL.add, op1=AL.add)
                        return run
                    return f1, f2

                def ffn_micro_list(b, c, i, ws_):
                    f1, f2 = ffn_micros(b, c, i)
                    return ([f1(m, ws_) for m in range(FFT)] +
                            [f2(g, kq, ws_) for g in range(2) for kq in range(12)])

                def ln_stage(b, which, ws_, c):
                    bb = ws_["bb"]
                    X = xt[b][cur[b]]
                    R = xt[b][1 - cur[b]]
                    if which == 1:
                        ln_chunk(R, X, bb[:, _L1G:_L1G + 6], bb[:, _L1N:_L1N + 6],
                                 bb[:, _CAB + b * 6:_CAB + (b + 1) * 6], c)
                    elif which == 2:
                        ln_chunk(X, R, bb[:, _L2G:_L2G + 6], bb[:, _L2N:_L2N + 6],
                                 bb[:, _L2B:_L2B + 6], c)
                    else:
                        ln_chunk(X, R, bb[:, _L3G:_L3G + 6], bb[:, _L3N:_L3N + 6],
                                 bb[:, _L3B:_L3B + 6], c)

                def head_chunk(b, c):
                    Xf = xt[b][cur[b]]
                    cs = slice(c * CH, (c + 1) * CH)
                    pf = pmm.tile([A_DIM, CH], F32, tag="mm")
                    for k in range(KT):
                        nc.tensor.matmul(pf[:], fct[:, k * A_DIM:(k + 1) * A_DIM],
                                         Xf[k][:, cs], start=(k == 0),
                                         stop=(k == KT - 1))
                    yt = scr.tile([A_DIM, CH], F32, tag="scr")
                    nc.vector.tensor_scalar_add(yt[:], pf[:], fcbt[:])
                    nc.sync.dma_start(y[b, :, cs], yt[:])

                # ---------- transformer blocks: 2-batch pipeline ----------
                WSn = None
                qkv_b = [None, None]   # live qkv tiles per batch
                # prologue: embed inputs stream first, then block-0 weights,
                # then b0's QKV (overlaps embed LN tails); late-needed consts
                # (mask, head weights) load last
                embed_chunk(0, 0)
                embed_chunk(0, 1)
                WS = wload(0)
                wload2(0, WS)
                embed_chunk(1, 0)
                embed_chunk(1, 1)
                qkv_b[0] = alloc_qkv()
                qkv_chunk(0, 0, WS, qkv_b[0])
                qkv_chunk(0, 1, WS, qkv_b[0])
                nc.sync.dma_start(bigm[:], masks[:])
                nc.sync.dma_start(fct[:], fcw[:])
                nc.sync.dma_start(fcbt[:],
                                  fcb[:].rearrange("(m o) -> m o", o=1))
                ot_b = [[None] * H, [None] * H]
                pend = None            # (b, ws, micros c0, micros c1) FFN of
                                       # other batch from previous block
                for i in range(NB):
                    for b in (0, 1):
                        o = 1 - b
                        if b == 0 and i > 0:
                            WS = WSn
                        if qkv_b[b] is None:  # prologue only (b0, block 0)
                            qkv_b[b] = alloc_qkv()
                            for c in range(NCH):
                                qkv_chunk(b, c, WS, qkv_b[b])
                        # attention zipped with other batch's pending FFN
                        if pend is not None:
                            pb, pws, pm0, pm1 = pend
                            att_chunk(b, 0, qkv_b[b], ot_b[b], pm0)
                            o_chunk(b, 0, WS, ot_b[b])
                            att_chunk(b, 1, qkv_b[b], ot_b[b], pm1)
                            ln_stage(pb, 3, pws, 0)
                            o_chunk(b, 1, WS, ot_b[b])
                            ln_stage(pb, 3, pws, 1)
                            cur[pb] = 1 - cur[pb]
                        else:
                            att_chunk(b, 0, qkv_b[b], ot_b[b], [])
                            o_chunk(b, 0, WS, ot_b[b])
                            att_chunk(b, 1, qkv_b[b], ot_b[b], [])
                            o_chunk(b, 1, WS, ot_b[b])
                        # LN1 zipped with other batch's QKV (next user of the
                        # single qkv tile set)
                        ln_stage(b, 1, WS, 0)
                        if b == 0:
                            # b1's qkv for this block
                            qkv_b[o] = alloc_qkv()
                            qkv_chunk(o, 0, WS, qkv_b[o])
                            ln_stage(b, 1, WS, 1)
                            qkv_chunk(o, 1, WS, qkv_b[o])
                            WSn = wload(i + 1) if i + 1 < NB else None
                        else:
                            if i + 1 < NB:
                                # b0's qkv for next block (uses next weights)
                                wload2(i + 1, WSn)
                                qkv_b[o] = alloc_qkv()
                                qkv_chunk(o, 0, WSn, qkv_b[o])
                                ln_stage(b, 1, WS, 1)
                                qkv_chunk(o, 1, WSn, qkv_b[o])
                            else:
                                ln_stage(b, 1, WS, 1)
                                qkv_b[o] = None
                        ln_stage(b, 2, WS, 0)
                        ln_stage(b, 2, WS, 1)
                        pend = (b, WS, ffn_micro_list(b, 0, i, WS),
                                ffn_micro_list(b, 1, i, WS))

                # ---------- epilogue: b1's last FFN + LN3, then head ----------
                pb, pws, pm0, pm1 = pend
                for z in pm0:
                    z()
                head_chunk(0, 0)
                for z in pm1:
                    z()
                head_chunk(0, 1)
                ln_stage(pb, 3, pws, 0)
                ln_stage(pb, 3, pws, 1)
                cur[1] = 1 - cur[1]
                head_chunk(1, 0)
                head_chunk(1, 1)

            for _rep in range(reps):
                emit_forward()

    nc.compile()
    return nc


def _posenc(length, d):
    pos_ = np.arange(length, dtype=np.float32)[:, None]
    i = np.arange(0, d, 2, dtype=np.float32)[None, :]
    ang = pos_ / np.power(np.float32(10000.0), i / np.float32(d))
    pe = np.zeros((length, d), np.float32)
    pe[:, 0::2] = np.sin(ang)
    pe[:, 1::2] = np.cos(ang)
    return pe


def _pack_pk(w, p=128):
    """[K*p, M] -> [p, K*M] partition-major packing."""
    k = w.shape[0] // p
    return np.ascontiguousarray(
        w.reshape(k, p, w.shape[1]).transpose(1, 0, 2).reshape(p, -1))


def _host_prep(inp):
    f32 = np.float32
    a, r, s, t = (np.asarray(inp[k]) for k in ("a", "r", "s", "t"))
    ars = np.concatenate(
        [np.asarray(a, f32), np.asarray(r, f32), np.asarray(s, f32)],
        axis=-1).transpose(0, 2, 1)  # [B, 193, L]
    ars = np.ascontiguousarray(ars).astype(bf)

    scale = f32(1.0 / np.sqrt(DH))
    sa_Wqkv = np.asarray(inp["sa_Wqkv"], f32)
    sa_bqkv = np.asarray(inp["sa_bqkv"], f32)
    wq_p = np.stack([_pack_pk((sa_Wqkv[i, 0] * scale).astype(bf))
                     for i in range(NB)])
    wk_p = np.stack([_pack_pk(sa_Wqkv[i, 1].astype(bf)) for i in range(NB)])
    wv_p = np.stack([_pack_pk(sa_Wqkv[i, 2].astype(bf)) for i in range(NB)])
    wo_p = np.stack([_pack_pk(np.asarray(inp["sa_Wo"], f32)[i].astype(bf), p=DH)
                     for i in range(NB)])
    w1_p = np.stack([
        np.ascontiguousarray(
            np.asarray(inp["ff_W1"], f32)[i].astype(bf)
            .reshape(KT, 128, FFT, 128).transpose(1, 2, 0, 3)
            .reshape(128, FFT, KT * 128))
        for i in range(NB)])
    w2_p = np.stack([
        _pack_pk(np.asarray(inp["ff_W2"], f32)[i].astype(bf)).reshape(
            128, FFT, D) for i in range(NB)])

    bq = sa_bqkv[:, 0] * scale
    bk = sa_bqkv[:, 1]
    bv = sa_bqkv[:, 2]
    bvb = np.zeros((NB, 128, 8 * 97), f32)
    for h in range(H):
        bvb[:, :, h * 97:h * 97 + DH] = bv[:, None, h * DH:(h + 1) * DH]
        bvb[:, :, h * 97 + DH] = 1.0
    pcol = np.arange(128)[:, None]
    ucol = np.arange(896)[None, :]
    masks = np.where(pcol > ucol - 384, f32(0.0), f32(1.0))

    task_table = np.asarray(inp["task_table"], f32)
    ca_Wqkv = np.asarray(inp["ca_Wqkv"], f32)
    ca_bqkv = np.asarray(inp["ca_bqkv"], f32)
    ca_Wo = np.asarray(inp["ca_Wo"], f32)
    ca_bo = np.asarray(inp["ca_bo"], f32)
    ln1_b = np.asarray(inp["ln1_b"], f32)
    enc = task_table[np.asarray(t)[:, 0]]  # [B, D]
    cab = np.zeros((NB, B, D), f32)
    for i in range(NB):
        v_ = enc @ ca_Wqkv[i, 2] + ca_bqkv[i, 2]
        cab[i] = v_ @ ca_Wo[i] + ca_bo[i]
    cabb_all = cab + ln1_b[:, None, :]  # [NB, B, D]

    def cols(x, p=128):
        # [n*p] -> [p, n] column pack
        return x.reshape(-1, p).T

    ln1_g = np.asarray(inp["ln1_g"], f32)
    l2g = np.asarray(inp["ln2_g"], f32)
    l2b = np.asarray(inp["ln2_b"], f32)
    l3g = np.asarray(inp["ln3_g"], f32)
    l3b = np.asarray(inp["ln3_b"], f32)
    bo_ = np.asarray(inp["sa_bo"], f32)
    b1_ = np.asarray(inp["ff_b1"], f32)
    b2_ = np.asarray(inp["ff_b2"], f32)

    bblk_shared = np.zeros((NB, 128, 112), f32)
    for i in range(NB):
        bblk_shared[i, 0:DH, _BQ:_BQ + 8] = bq[i].reshape(H, DH).T
        bblk_shared[i, 0:DH, _BK:_BK + 8] = bk[i].reshape(H, DH).T
        bblk_shared[i, :, _BO:_BO + 6] = cols(bo_[i])
        bblk_shared[i, :, _B1:_B1 + 24] = cols(b1_[i])
        bblk_shared[i, :, _B2:_B2 + 6] = cols(b2_[i])
        bblk_shared[i, :, _L1G:_L1G + 6] = cols(ln1_g[i])
        bblk_shared[i, :, _L1N:_L1N + 6] = cols(-ln1_g[i])
        bblk_shared[i, :, _L2G:_L2G + 6] = cols(l2g[i])
        bblk_shared[i, :, _L2N:_L2N + 6] = cols(-l2g[i])
        bblk_shared[i, :, _L2B:_L2B + 6] = cols(l2b[i])
        bblk_shared[i, :, _L3G:_L3G + 6] = cols(l3g[i])
        bblk_shared[i, :, _L3N:_L3N + 6] = cols(-l3g[i])
        bblk_shared[i, :, _L3B:_L3B + 6] = cols(l3b[i])

    ln_g = np.asarray(inp["ln_g"], f32)
    ln_b = np.asarray(inp["ln_b"], f32)
    bemb = np.concatenate([np.asarray(inp["ba"], f32),
                           np.asarray(inp["br"], f32),
                           np.asarray(inp["bs"], f32)])
    emb0 = np.zeros((128, 24), f32)
    emb0[:, 0:6] = cols(bemb)
    emb0[:, 6:12] = cols(ln_g)
    emb0[:, 12:18] = cols(-ln_g)
    emb0[:, 18:24] = cols(ln_b)

    wab = np.concatenate([np.asarray(inp["Wa"], f32),
                          np.asarray(inp["Wr"], f32)], axis=0)  # [65, E]
    fcw_p = _pack_pk(np.asarray(inp["fc_W"], f32).astype(bf))  # [128, 6*64]

    shared = dict(
        wab=wab.astype(bf),
        wsd=np.asarray(inp["Ws"], f32).astype(bf),
        emb0=emb0,
        pos=np.ascontiguousarray(_posenc(L, D).T
                                 + ln_b[:, None]),
        wq=wq_p, wk=wk_p, wv=wv_p, wo=wo_p, w1=w1_p, w2=w2_p,
        bvb=bvb.astype(bf),
        masks=masks.astype(bf),
        fcw=fcw_p,
        fcb=np.asarray(inp["fc_b"], f32),
    )
    in_maps = []
    for core in range(NCORES):
        m = dict(shared)
        m["ars"] = ars[core * CPC:(core + 1) * CPC]
        bb = bblk_shared.copy()
        for i in range(NB):
            for b in range(CPC):
                bb[i, :, _CAB + b * 6:_CAB + (b + 1) * 6] = cols(
                    cabb_all[i, core * CPC + b])
        m["bblk"] = bb
        in_maps.append(m)
    return in_maps


def _get_nc(reps=1):
    key = f"nc{reps}"
    if key not in _CACHE:
        _CACHE[key] = _build(reps)
    return _CACHE[key]


def kernel(**inputs):
    nc = _get_nc()
    in_maps = _host_prep(inputs)
    res = run_bass_kernel_spmd(nc, in_maps, core_ids=list(range(NCORES)))
    out = np.zeros((B, L, A_DIM), np.float32)
    for core in range(NCORES):
        yc = res.results[core]["y"]  # [CPC, 64, L]
        for b in range(CPC):
            out[core * CPC + b] = yc[b].T
    return out
